# revision 7
# baseline (speedup 1.0000x reference)
"""Trainium2 Bass kernel for random-tensor-product spherical harmonics.

Math: for each 6-D coordinate row, split into two 3-vectors o1, o2. For each,
real spherical harmonics Y_lm up to l=20 (K=441), then
out[n, r] = Y1[n, rand_i[r]] * Y2[n, rand_j[r]]  (RANK=256).

Factorization used on-device, per sphere (unit vector u,v,w; s=sin(theta),
cos(phi)=u/s, sin(phi)=v/s):
  Y_lm = [s^(m mod 2) * trig_m(phi)] * [poly_{l,m}(w)]
where trig_m = cos(m phi) (m>=0) or sin(|m| phi) (m<0), and
poly_{l,m}(w) = norm * (1-w^2)^floor(|m|/2) * (P_l^|m|(w)/s^|m|) is a
polynomial of degree <= 20 in w, evaluated in the Chebyshev basis T_k(w).

Device features (62 rows, computed point-major with DVE/GPSIMD/ACT):
  row 0: ones; rows 2m-1 / 2m (m=1..20): s^(m mod 2)-folded cos(m phi) /
  sin(m phi) from the complex-power recurrence; rows 41+k: T_k(w).
Per 128-point chunk: PE-transpose features -> [124,128] lhsT (both spheres),
two matmuls against host-built coefficient matrices give
  [TRIG1 | POLY1] and [TRIG2 | POLY2] in PSUM; then
  out = (TRIG1*POLY1) * (TRIG2*POLY2) via DVE (PSUM-legal muls) + GPSIMD.

Sharding: data-parallel over points across 8 NeuronCores; rand_i/rand_j are
folded into the replicated coefficient matrices on the host.
"""
import math

import numpy as np

import concourse.bass as bass
import concourse.tile as tile
from concourse import mybir
from concourse.bass_utils import run_bass_kernel_spmd

F32 = mybir.dt.float32
F32R = mybir.dt.float32r
AF = mybir.ActivationFunctionType
OP = mybir.AluOpType

N_CORES = 8
N = 262144
NPC = N // N_CORES          # 32768 points per core
P = 128                     # partitions
C = NPC // P                # 256 chunks (free columns) per core
RANK = 256
MAX_DEGREE = 20
NTRIG = 41                  # ones + (cos,sin) x m=1..20
NPOLY = 21                  # T_0..T_20
NFEAT = NTRIG + NPOLY       # 62
MM_K = 2 * NFEAT            # 124 matmul contract rows (feat x sphere)
MM_DTYPE = F32R             # fp32r: 1 cy/row at N=512; ~1e-4 rel error

GROUP = 4                   # chunks per output group (PSUM/DVE batching)

_MAX_WAITS = 1


def _split_sync_waits(nc):
    """walrus in this container rejects >1 sync wait per instruction; hoist
    excess waits onto InstNoOps inserted before the instruction."""
    counter = [0]

    def fresh_nop(engine, waits):
        counter[0] += 1
        nop = mybir.InstNoOp(name=f"Wsplit-{counter[0]}", ins=[], outs=[])
        nop.engine = engine
        nop.sync_info = mybir.SyncInfo(on_wait=list(waits), on_update=[])
        return nop

    for f in nc.m.functions:
        for bb in f.blocks:
            insts = bb.instructions
            new = []
            changed = False
            for inst in insts:
                si = getattr(inst, "sync_info", None)
                if si is not None and si.on_wait and len(si.on_wait) > _MAX_WAITS:
                    waits = list(si.on_wait)
                    rest, keep = waits[:-_MAX_WAITS], waits[-_MAX_WAITS:]
                    while rest:
                        new.append(fresh_nop(inst.engine, rest[:_MAX_WAITS]))
                        rest = rest[_MAX_WAITS:]
                    si.on_wait = keep
                    inst.sync_info = si
                    changed = True
                new.append(inst)
            if changed:
                bb.instructions = new


# ---------------------------------------------------------------------------
# Host-side coefficient construction
# ---------------------------------------------------------------------------

def _legendre_q(l, m, x):
    """Q_{l,m}(x) = P_l^m(x)/s^m (reference recurrence with Condon-Shortley),
    evaluated in float64. x: ndarray."""
    q_prev = np.ones_like(x)            # Q[m',m'] chain
    for mp in range(1, m + 1):
        q_prev = -(2 * mp - 1) * q_prev
    if l == m:
        return q_prev * np.ones_like(x)
    q_mm = q_prev * np.ones_like(x)
    q_next = (2 * m + 1) * x * q_mm     # Q[m+1,m]
    if l == m + 1:
        return q_next
    qa, qb = q_mm, q_next
    for ll in range(m + 2, l + 1):
        qc = ((2 * ll - 1) * x * qb - (ll + m - 1) * qa) / (ll - m)
        qa, qb = qb, qc
    return qb


def _poly_cheb_coeffs(l, m_abs, m_is_zero):
    """Chebyshev coefficients (degree<=20) of
    norm_eff * (1-w^2)^floor(m/2) * Q_{l,m}(w)."""
    norm = math.sqrt((2 * l + 1) / (4.0 * math.pi)
                     * math.factorial(l - m_abs) / math.factorial(l + m_abs))
    if not m_is_zero:
        norm *= math.sqrt(2.0)
    t = np.arange(21, dtype=np.float64)
    w = np.cos(np.pi * (t + 0.5) / 21.0)      # Chebyshev nodes
    vals = norm * (1.0 - w * w) ** (m_abs // 2) * _legendre_q(l, m_abs, w)
    return np.polynomial.chebyshev.chebfit(w, vals, MAX_DEGREE)


def _build_rhs(rand_i, rand_j):
    """Two [MM_K, 512] f32 matrices.

    rhsT: cols 0:256 one-hot trig gather for sphere 1 (rows 2*f), cols
    256:512 for sphere 2 (rows 2*f+1).  rhsP: same column split with the
    poly Chebyshev coefficients.  So MM outputs are [TRIG1|TRIG2] and
    [POLY1|POLY2]."""
    rhsT = np.zeros((MM_K, 2 * RANK), dtype=np.float64)
    rhsP = np.zeros((MM_K, 2 * RANK), dtype=np.float64)
    for sphere, rand_idx in ((0, rand_i), (1, rand_j)):
        for r in range(RANK):
            idx = int(rand_idx[r])
            l = int(math.isqrt(idx))
            m = idx - l * l - l
            m_abs = abs(m)
            if m == 0:
                trig_row = 0
            elif m > 0:
                trig_row = 2 * m - 1          # cos row
            else:
                trig_row = 2 * m_abs          # sin row
            rhsT[2 * trig_row + sphere, sphere * RANK + r] = 1.0
            coeffs = _poly_cheb_coeffs(l, m_abs, m == 0)
            for k in range(MAX_DEGREE + 1):
                rhsP[2 * (NTRIG + k) + sphere, sphere * RANK + r] = coeffs[k]
    return rhsT.astype(np.float32), rhsP.astype(np.float32)


# ---------------------------------------------------------------------------
# Device kernel
# ---------------------------------------------------------------------------

def _build_nc():
    nc = bass.Bass("TRN2", target_bir_lowering=False, debug=False)
    coords = nc.declare_dram_parameter("coords", [NPC, 6], F32, isOutput=False)
    rhsA = nc.declare_dram_parameter("rhsA", [MM_K, 2 * RANK], MM_DTYPE,
                                     isOutput=False)
    rhsB = nc.declare_dram_parameter("rhsB", [MM_K, 2 * RANK], MM_DTYPE,
                                     isOutput=False)
    ident = nc.declare_dram_parameter("ident", [P, P], F32, isOutput=False)
    out = nc.declare_dram_parameter("out", [NPC, RANK], F32, isOutput=True)

    # DRAM views
    coords_v = coords.rearrange("(p c) d -> p (c d)", p=P)       # [128, 1536]
    out_v = out.rearrange("(p c) r -> p c r", p=P)               # [128, 256, 256]

    with tile.TileContext(nc) as tc:
        with (
            tc.tile_pool(name="const", bufs=1) as constp,
            tc.tile_pool(name="feat", bufs=1) as featp,
            tc.tile_pool(name="setup", bufs=1) as setupp,
        ):
            # ---- constants -------------------------------------------------
            rA = constp.tile([MM_K, 2 * RANK], MM_DTYPE)
            nc.sync.dma_start(rA[:], rhsA[:])
            rB = constp.tile([MM_K, 2 * RANK], MM_DTYPE)
            nc.sync.dma_start(rB[:], rhsB[:])
            idt = constp.tile([P, P], F32)
            nc.sync.dma_start(idt[:], ident[:])

            # ---- load + deinterleave coordinates --------------------------
            raw = setupp.tile([P, C * 6], F32)
            nc.sync.dma_start(raw[:], coords_v[:])
            raw3 = raw[:].rearrange("p (c d) -> p c d", d=6)
            # both-sphere tiles [128, 512]: cols 0:256 sphere1, 256:512 sphere2
            FD = 2 * C
            xb = setupp.tile([P, FD], F32)
            yb = setupp.tile([P, FD], F32)
            zb = setupp.tile([P, FD], F32)
            for s in range(2):
                nc.vector.tensor_copy(xb[:, s * C:(s + 1) * C], raw3[:, :, 3 * s])
                nc.vector.tensor_copy(yb[:, s * C:(s + 1) * C], raw3[:, :, 3 * s + 1])
                nc.vector.tensor_copy(zb[:, s * C:(s + 1) * C], raw3[:, :, 3 * s + 2])

            # ---- setup: unit vector, s, cos/sin(phi) ----------------------
            x2 = setupp.tile([P, FD], F32)
            nc.vector.tensor_mul(x2[:], xb[:], xb[:])
            y2 = setupp.tile([P, FD], F32)
            nc.vector.tensor_mul(y2[:], yb[:], yb[:])
            rho2 = setupp.tile([P, FD], F32)
            nc.vector.tensor_add(rho2[:], x2[:], y2[:])
            z2 = setupp.tile([P, FD], F32)
            nc.vector.tensor_mul(z2[:], zb[:], zb[:])
            r2 = setupp.tile([P, FD], F32)
            nc.vector.tensor_add(r2[:], rho2[:], z2[:])
            # guard rho2 against exact zero (atan2(0,0) corner)
            rho2g = setupp.tile([P, FD], F32)
            nc.vector.tensor_scalar_max(rho2g[:], rho2[:], 1e-30)
            rr = setupp.tile([P, FD], F32)
            nc.scalar.activation(rr[:], r2[:], AF.Sqrt)
            rho = setupp.tile([P, FD], F32)
            nc.scalar.activation(rho[:], rho2g[:], AF.Sqrt)
            rinv = setupp.tile([P, FD], F32)
            nc.vector.reciprocal(rinv[:], rr[:])
            rhoinv = setupp.tile([P, FD], F32)
            nc.vector.reciprocal(rhoinv[:], rho[:])
            w = setupp.tile([P, FD], F32)
            nc.vector.tensor_mul(w[:], zb[:], rinv[:])
            cphi = setupp.tile([P, FD], F32)
            nc.vector.tensor_mul(cphi[:], xb[:], rhoinv[:])
            sphi = setupp.tile([P, FD], F32)
            nc.vector.tensor_mul(sphi[:], yb[:], rhoinv[:])
            ssin = setupp.tile([P, FD], F32)
            nc.vector.tensor_mul(ssin[:], rho[:], rinv[:])

            # ---- feature region [128, NFEAT, 512] -------------------------
            feat = featp.tile([P, NFEAT, FD], F32)

            def frow(j):
                return feat[:, j, :]

            nc.gpsimd.memset(frow(0), 1.0)            # trig m=0
            nc.gpsimd.memset(frow(NTRIG), 1.0)        # T_0

            # Chebyshev chain on GPSIMD: T_1 = w; T_k = 2w*T_{k-1} - T_{k-2}
            w2 = setupp.tile([P, FD], F32)
            nc.vector.tensor_add(w2[:], w[:], w[:])
            nc.gpsimd.tensor_copy(frow(NTRIG + 1), w[:])
            with tc.tile_pool(name="chsc", bufs=2) as chsc:
                for k in range(2, MAX_DEGREE + 1):
                    tkt = chsc.tile([P, FD], F32, tag="chebt")
                    nc.gpsimd.tensor_mul(tkt[:], w2[:], frow(NTRIG + k - 1))
                    nc.gpsimd.tensor_sub(frow(NTRIG + k), tkt[:], frow(NTRIG + k - 2))

            # trig chain on DVE; odd-m rows folded by s on GPSIMD
            with tc.tile_pool(name="trsc", bufs=2) as trsc:
                # m=1 (odd): folded rows
                nc.gpsimd.tensor_mul(frow(1), cphi[:], ssin[:])
                nc.gpsimd.tensor_mul(frow(2), sphi[:], ssin[:])
                cm_prev, sm_prev = cphi, sphi      # unfolded c_{m-1}, s_{m-1}
                for m in range(2, MAX_DEGREE + 1):
                    t1 = trsc.tile([P, FD], F32, tag="t1")
                    nc.vector.tensor_mul(t1[:], cm_prev[:], cphi[:])
                    t2 = trsc.tile([P, FD], F32, tag="t2")
                    nc.vector.tensor_mul(t2[:], sm_prev[:], sphi[:])
                    t3 = trsc.tile([P, FD], F32, tag="t3")
                    nc.vector.tensor_mul(t3[:], sm_prev[:], cphi[:])
                    t4 = trsc.tile([P, FD], F32, tag="t4")
                    nc.vector.tensor_mul(t4[:], cm_prev[:], sphi[:])
                    if m % 2 == 0:
                        # even: unfolded values live in the feature rows
                        nc.vector.tensor_sub(frow(2 * m - 1), t1[:], t2[:])
                        nc.vector.tensor_add(frow(2 * m), t3[:], t4[:])
                        cm_prev = feat[:, 2 * m - 1, :]
                        sm_prev = feat[:, 2 * m, :]
                    else:
                        cmo = trsc.tile([P, FD], F32, tag="cmo")
                        nc.vector.tensor_sub(cmo[:], t1[:], t2[:])
                        smo = trsc.tile([P, FD], F32, tag="smo")
                        nc.vector.tensor_add(smo[:], t3[:], t4[:])
                        nc.gpsimd.tensor_mul(frow(2 * m - 1), cmo[:], ssin[:])
                        nc.gpsimd.tensor_mul(frow(2 * m), smo[:], ssin[:])
                        cm_prev, sm_prev = cmo, smo

            # ---- per-chunk pipeline ---------------------------------------
            with (
                tc.tile_pool(name="pst", bufs=2, space="PSUM") as pstp,
                tc.tile_pool(name="psT", bufs=2, space="PSUM") as psTp,
                tc.tile_pool(name="psP", bufs=2, space="PSUM") as psPp,
                tc.tile_pool(name="phi", bufs=3) as phip,
                tc.tile_pool(name="trigc", bufs=4) as trigcp,
                tc.tile_pool(name="gfac", bufs=2) as gfacp,
                tc.tile_pool(name="outp", bufs=2) as outp,
            ):
                for g in range(C // GROUP):
                    # per-chunk [TRIG1*POLY1 | TRIG2*POLY2] pairs for GROUP chunks
                    gp = gfacp.tile([P, GROUP, 2 * RANK], F32)
                    for cc in range(GROUP):
                        c = g * GROUP + cc
                        # transpose features for chunk c: [128,124] -> [124,128]
                        fin = feat[:, :, c::C]            # [128, 62, 2]
                        pht = pstp.tile([MM_K, P], F32)
                        nc.tensor.transpose(pht[:], fin, idt[:])
                        phs = phip.tile([MM_K, P], MM_DTYPE)
                        nc.scalar.copy(phs[:], pht[:])    # cast to f32r
                        pT = psTp.tile([P, 2 * RANK], F32)
                        nc.tensor.matmul(pT[:], phs[:], rA[:], start=True, stop=True)
                        pP = psPp.tile([P, 2 * RANK], F32)
                        nc.tensor.matmul(pP[:], phs[:], rB[:], start=True, stop=True)
                        tc_sb = trigcp.tile([P, 2 * RANK], F32)
                        nc.scalar.copy(tc_sb[:], pT[:])
                        nc.vector.tensor_mul(gp[:, cc, :], tc_sb[:], pP[:])
                    ot = outp.tile([P, GROUP * RANK], F32)
                    otv = ot[:].rearrange("p (c r) -> p c r", r=RANK)
                    nc.gpsimd.tensor_mul(otv, gp[:, :, 0:RANK],
                                         gp[:, :, RANK:2 * RANK])
                    nc.sync.dma_start(
                        out_v[:, g * GROUP:(g + 1) * GROUP, :], ot[:])

    _split_sync_waits(nc)
    return nc


_CACHE = {}


def _install_ntff_shim():
    """Provide antenv.axon_hooks (absent in this image) so that
    run_bass_kernel_spmd(trace=True) can NTFF-profile via the axon .so."""
    import contextlib
    import ctypes
    import sys
    import types

    if "antenv.axon_hooks" in sys.modules:
        return
    so_path = "/opt/axon/libaxon_pjrt.so"
    lib = ctypes.CDLL(so_path)
    lib.axon_start_nrt_profile.argtypes = [
        ctypes.POINTER(ctypes.c_int64), ctypes.c_size_t]
    lib.axon_start_nrt_profile.restype = ctypes.c_int64
    lib.axon_stop_nrt_profile.argtypes = [ctypes.c_char_p]
    lib.axon_stop_nrt_profile.restype = ctypes.c_int64

    @contextlib.contextmanager
    def _hook(output_dir, device_ids):
        import jax
        jax.devices()
        if device_ids:
            ids = (ctypes.c_int64 * len(device_ids))(*device_ids)
            rc = lib.axon_start_nrt_profile(ids, len(device_ids))
        else:
            rc = lib.axon_start_nrt_profile(None, 0)
        if rc != 0:
            raise RuntimeError(f"axon_start_nrt_profile rc={rc}")
        try:
            yield
        finally:
            n = lib.axon_stop_nrt_profile(str(output_dir).encode())
            print(f"ntff profile: {n} file(s) written to {output_dir}")

    mod = types.ModuleType("antenv.axon_hooks")
    mod.get_axon_ntff_profile_hook = lambda: _hook
    mod.set_axon_ntff_profile_hook = lambda h: None
    sys.modules["antenv.axon_hooks"] = mod


def kernel(coordinates, rand_i, rand_j):
    assert coordinates.shape == (N, 6)
    rhsA, rhsB = _build_rhs(np.asarray(rand_i), np.asarray(rand_j))
    identity = np.eye(P, dtype=np.float32)

    if "nc" not in _CACHE:
        _CACHE["nc"] = _build_nc()
    nc = _CACHE["nc"]

    coords = np.ascontiguousarray(coordinates, dtype=np.float32)
    in_maps = []
    for i in range(N_CORES):
        in_maps.append({
            "coords": coords[i * NPC:(i + 1) * NPC],
            "rhsA": rhsA,
            "rhsB": rhsB,
            "ident": identity,
        })
    import os
    trace = bool(os.environ.get("KERNEL_TRACE"))
    if trace:
        _install_ntff_shim()
    res = run_bass_kernel_spmd(nc, in_maps, core_ids=list(range(N_CORES)),
                               trace=trace)
    if trace:
        _CACHE["last_exec_time_ns"] = res.exec_time_ns
        _CACHE["last_profile"] = res
    out = np.concatenate([res.results[i]["out"] for i in range(N_CORES)], axis=0)
    return out.astype(np.float32)


# revision 10
# speedup vs baseline: 1.0935x; 1.0935x over previous
"""Trainium2 Bass kernel for random-tensor-product spherical harmonics.

Math: for each 6-D coordinate row, split into two 3-vectors o1, o2. For each,
real spherical harmonics Y_lm up to l=20 (K=441), then
out[n, r] = Y1[n, rand_i[r]] * Y2[n, rand_j[r]]  (RANK=256).

Factorization used on-device, per sphere (unit vector u,v,w; s=sin(theta),
cos(phi)=u/s, sin(phi)=v/s):
  Y_lm = [s^(m mod 2) * trig_m(phi)] * [poly_{l,m}(w)]
where trig_m = cos(m phi) (m>=0) or sin(|m| phi) (m<0), and
poly_{l,m}(w) = norm * (1-w^2)^floor(|m|/2) * (P_l^|m|(w)/s^|m|) is a
polynomial of degree <= 20 in w, evaluated in the Chebyshev basis T_k(w).

Device features (62 rows, computed point-major with DVE/GPSIMD/ACT):
  row 0: ones; rows 2m-1 / 2m (m=1..20): s^(m mod 2)-folded cos(m phi) /
  sin(m phi) from the complex-power recurrence; rows 41+k: T_k(w).
Per 128-point chunk: PE-transpose features -> [124,128] lhsT (both spheres),
two matmuls against host-built coefficient matrices give
  [TRIG1 | POLY1] and [TRIG2 | POLY2] in PSUM; then
  out = (TRIG1*POLY1) * (TRIG2*POLY2) via DVE (PSUM-legal muls) + GPSIMD.

Sharding: data-parallel over points across 8 NeuronCores; rand_i/rand_j are
folded into the replicated coefficient matrices on the host.
"""
import math

import numpy as np

import concourse.bass as bass
import concourse.tile as tile
from concourse import mybir
from concourse.bass_utils import run_bass_kernel_spmd

F32 = mybir.dt.float32
F32R = mybir.dt.float32r
AF = mybir.ActivationFunctionType
OP = mybir.AluOpType

N_CORES = 8
N = 262144
NPC = N // N_CORES          # 32768 points per core
P = 128                     # partitions
C = NPC // P                # 256 chunks (free columns) per core
RANK = 256
MAX_DEGREE = 20
NTRIG = 41                  # ones + (cos,sin) x m=1..20
NPOLY = 21                  # T_0..T_20
NFEAT = NTRIG + NPOLY       # 62
MM_K = 2 * NFEAT            # 124 matmul contract rows (feat x sphere)
MM_DTYPE = F32R             # fp32r: 1 cy/row at N=512; ~1e-4 rel error

GROUP = 4                   # chunks per output group (PSUM/DVE batching)

_MAX_WAITS = 1


def _split_sync_waits(nc):
    """walrus in this container rejects >1 sync wait per instruction; hoist
    excess waits onto InstNoOps inserted before the instruction."""
    counter = [0]

    def fresh_nop(engine, waits):
        counter[0] += 1
        nop = mybir.InstNoOp(name=f"Wsplit-{counter[0]}", ins=[], outs=[])
        nop.engine = engine
        nop.sync_info = mybir.SyncInfo(on_wait=list(waits), on_update=[])
        return nop

    for f in nc.m.functions:
        for bb in f.blocks:
            insts = bb.instructions
            new = []
            changed = False
            for inst in insts:
                si = getattr(inst, "sync_info", None)
                if si is not None and si.on_wait and len(si.on_wait) > _MAX_WAITS:
                    waits = list(si.on_wait)
                    rest, keep = waits[:-_MAX_WAITS], waits[-_MAX_WAITS:]
                    while rest:
                        new.append(fresh_nop(inst.engine, rest[:_MAX_WAITS]))
                        rest = rest[_MAX_WAITS:]
                    si.on_wait = keep
                    inst.sync_info = si
                    changed = True
                new.append(inst)
            if changed:
                bb.instructions = new


# ---------------------------------------------------------------------------
# Host-side coefficient construction
# ---------------------------------------------------------------------------

def _legendre_q(l, m, x):
    """Q_{l,m}(x) = P_l^m(x)/s^m (reference recurrence with Condon-Shortley),
    evaluated in float64. x: ndarray."""
    q_prev = np.ones_like(x)            # Q[m',m'] chain
    for mp in range(1, m + 1):
        q_prev = -(2 * mp - 1) * q_prev
    if l == m:
        return q_prev * np.ones_like(x)
    q_mm = q_prev * np.ones_like(x)
    q_next = (2 * m + 1) * x * q_mm     # Q[m+1,m]
    if l == m + 1:
        return q_next
    qa, qb = q_mm, q_next
    for ll in range(m + 2, l + 1):
        qc = ((2 * ll - 1) * x * qb - (ll + m - 1) * qa) / (ll - m)
        qa, qb = qb, qc
    return qb


def _poly_cheb_coeffs(l, m_abs, m_is_zero):
    """Chebyshev coefficients (degree<=20) of
    norm_eff * (1-w^2)^floor(m/2) * Q_{l,m}(w)."""
    norm = math.sqrt((2 * l + 1) / (4.0 * math.pi)
                     * math.factorial(l - m_abs) / math.factorial(l + m_abs))
    if not m_is_zero:
        norm *= math.sqrt(2.0)
    t = np.arange(21, dtype=np.float64)
    w = np.cos(np.pi * (t + 0.5) / 21.0)      # Chebyshev nodes
    vals = norm * (1.0 - w * w) ** (m_abs // 2) * _legendre_q(l, m_abs, w)
    return np.polynomial.chebyshev.chebfit(w, vals, MAX_DEGREE)


def _build_rhs(rand_i, rand_j):
    """Two [MM_K, 512] f32 matrices.

    rhsT: cols 0:256 one-hot trig gather for sphere 1 (rows 2*f), cols
    256:512 for sphere 2 (rows 2*f+1).  rhsP: same column split with the
    poly Chebyshev coefficients.  So MM outputs are [TRIG1|TRIG2] and
    [POLY1|POLY2]."""
    rhsT = np.zeros((MM_K, 2 * RANK), dtype=np.float64)
    rhsP = np.zeros((MM_K, 2 * RANK), dtype=np.float64)
    for sphere, rand_idx in ((0, rand_i), (1, rand_j)):
        for r in range(RANK):
            idx = int(rand_idx[r])
            l = int(math.isqrt(idx))
            m = idx - l * l - l
            m_abs = abs(m)
            if m == 0:
                trig_row = 0
            elif m > 0:
                trig_row = 2 * m - 1          # cos row
            else:
                trig_row = 2 * m_abs          # sin row
            rhsT[2 * trig_row + sphere, sphere * RANK + r] = 1.0
            coeffs = _poly_cheb_coeffs(l, m_abs, m == 0)
            for k in range(MAX_DEGREE + 1):
                rhsP[2 * (NTRIG + k) + sphere, sphere * RANK + r] = coeffs[k]
    return rhsT.astype(np.float32), rhsP.astype(np.float32)


# ---------------------------------------------------------------------------
# Device kernel
# ---------------------------------------------------------------------------

def _build_nc():
    nc = bass.Bass("TRN2", target_bir_lowering=False, debug=False)
    coords = nc.declare_dram_parameter("coords", [NPC, 6], F32, isOutput=False)
    rhsA = nc.declare_dram_parameter("rhsA", [MM_K, 2 * RANK], MM_DTYPE,
                                     isOutput=False)
    rhsB = nc.declare_dram_parameter("rhsB", [MM_K, 2 * RANK], MM_DTYPE,
                                     isOutput=False)
    ident = nc.declare_dram_parameter("ident", [P, P], F32, isOutput=False)
    out = nc.declare_dram_parameter("out", [NPC, RANK], F32, isOutput=True)

    # DRAM views
    coords_v = coords.rearrange("(p c) d -> p (c d)", p=P)       # [128, 1536]
    out_v = out.rearrange("(p c) r -> p c r", p=P)               # [128, 256, 256]

    with tile.TileContext(nc) as tc:
        with (
            tc.tile_pool(name="const", bufs=1) as constp,
            tc.tile_pool(name="feat", bufs=1) as featp,
            tc.tile_pool(name="setup", bufs=1) as setupp,
        ):
            # ---- constants -------------------------------------------------
            rA = constp.tile([MM_K, 2 * RANK], MM_DTYPE)
            nc.sync.dma_start(rA[:], rhsA[:])
            rB = constp.tile([MM_K, 2 * RANK], MM_DTYPE)
            nc.sync.dma_start(rB[:], rhsB[:])
            idt = constp.tile([P, P], F32)
            nc.sync.dma_start(idt[:], ident[:])

            # ---- coordinates + feature region -----------------------------
            raw = setupp.tile([P, C * 6], F32)
            nc.sync.dma_start(raw[:], coords_v[:])
            raw3 = raw[:].rearrange("p (c d) -> p c d", d=6)
            FD = 2 * C          # both-sphere free dim: sphere1 | sphere2
            feat = featp.tile([P, NFEAT, FD], F32)

            NH = 2              # feature halves (chunk ranges) for overlap
            CH = C // NH

            def build_features(h):
                """Features for chunks [h*CH, (h+1)*CH) of both spheres.
                Per-op views are [128, 2, CH] (cols {c}+{C+c})."""
                lo = h * CH

                def half(t):
                    return t[:].rearrange("p (s c) -> p s c", s=2)[:, :, lo:lo + CH]

                def frow(j):
                    return feat[:, j, :].rearrange("p (s c) -> p s c", s=2)[:, :, lo:lo + CH]

                hFD = 2 * CH
                xb = setupp.tile([P, hFD], F32, tag="xb")
                yb = setupp.tile([P, hFD], F32, tag="yb")
                zb = setupp.tile([P, hFD], F32, tag="zb")
                for s in range(2):
                    nc.vector.tensor_copy(xb[:, s * CH:(s + 1) * CH],
                                          raw3[:, lo:lo + CH, 3 * s])
                    nc.vector.tensor_copy(yb[:, s * CH:(s + 1) * CH],
                                          raw3[:, lo:lo + CH, 3 * s + 1])
                    nc.vector.tensor_copy(zb[:, s * CH:(s + 1) * CH],
                                          raw3[:, lo:lo + CH, 3 * s + 2])
                x2 = setupp.tile([P, hFD], F32, tag="x2")
                nc.vector.tensor_mul(x2[:], xb[:], xb[:])
                y2 = setupp.tile([P, hFD], F32, tag="y2")
                nc.vector.tensor_mul(y2[:], yb[:], yb[:])
                rho2 = setupp.tile([P, hFD], F32, tag="rho2")
                nc.vector.tensor_add(rho2[:], x2[:], y2[:])
                z2 = setupp.tile([P, hFD], F32, tag="z2")
                nc.vector.tensor_mul(z2[:], zb[:], zb[:])
                r2 = setupp.tile([P, hFD], F32, tag="r2")
                nc.vector.tensor_add(r2[:], rho2[:], z2[:])
                rho2g = setupp.tile([P, hFD], F32, tag="rho2g")
                nc.vector.tensor_scalar_max(rho2g[:], rho2[:], 1e-30)
                rr = setupp.tile([P, hFD], F32, tag="rr")
                nc.scalar.activation(rr[:], r2[:], AF.Sqrt)
                rho = setupp.tile([P, hFD], F32, tag="rho")
                nc.scalar.activation(rho[:], rho2g[:], AF.Sqrt)
                rinv = setupp.tile([P, hFD], F32, tag="rinv")
                nc.vector.reciprocal(rinv[:], rr[:])
                rhoinv = setupp.tile([P, hFD], F32, tag="rhoinv")
                nc.vector.reciprocal(rhoinv[:], rho[:])
                w = setupp.tile([P, hFD], F32, tag="w")
                nc.vector.tensor_mul(w[:], zb[:], rinv[:])
                cphi = setupp.tile([P, hFD], F32, tag="cphi")
                nc.vector.tensor_mul(cphi[:], xb[:], rhoinv[:])
                sphi = setupp.tile([P, hFD], F32, tag="sphi")
                nc.vector.tensor_mul(sphi[:], yb[:], rhoinv[:])
                ssin = setupp.tile([P, hFD], F32, tag="ssin")
                nc.vector.tensor_mul(ssin[:], rho[:], rinv[:])

                nc.gpsimd.memset(frow(0), 1.0)            # trig m=0
                nc.gpsimd.memset(frow(NTRIG), 1.0)        # T_0

                # Chebyshev chain on GPSIMD
                w2 = setupp.tile([P, hFD], F32, tag="w2")
                nc.vector.tensor_add(w2[:], w[:], w[:])
                nc.gpsimd.tensor_copy(frow(NTRIG + 1), w[:])
                for k in range(2, MAX_DEGREE + 1):
                    tkt = setupp.tile([P, hFD], F32, tag="chebt")
                    nc.gpsimd.tensor_mul(tkt[:], w2[:], frow(NTRIG + k - 1))
                    nc.gpsimd.tensor_sub(frow(NTRIG + k), tkt[:],
                                         frow(NTRIG + k - 2))

                # trig chain on DVE; odd-m rows folded by s on GPSIMD
                nc.gpsimd.tensor_mul(frow(1), cphi[:], ssin[:])
                nc.gpsimd.tensor_mul(frow(2), sphi[:], ssin[:])
                cm_prev, sm_prev = cphi[:], sphi[:]
                for m in range(2, MAX_DEGREE + 1):
                    t1 = setupp.tile([P, hFD], F32, tag="t1")
                    nc.vector.tensor_mul(t1[:], cm_prev, cphi[:])
                    t2 = setupp.tile([P, hFD], F32, tag="t2")
                    nc.vector.tensor_mul(t2[:], sm_prev, sphi[:])
                    t3 = setupp.tile([P, hFD], F32, tag="t3")
                    nc.vector.tensor_mul(t3[:], sm_prev, cphi[:])
                    t4 = setupp.tile([P, hFD], F32, tag="t4")
                    nc.vector.tensor_mul(t4[:], cm_prev, sphi[:])
                    if m % 2 == 0:
                        nc.vector.tensor_sub(frow(2 * m - 1), t1[:], t2[:])
                        nc.vector.tensor_add(frow(2 * m), t3[:], t4[:])
                        cm_prev = frow(2 * m - 1)
                        sm_prev = frow(2 * m)
                    else:
                        cmo = setupp.tile([P, hFD], F32, tag=f"cmo{m % 4}")
                        nc.vector.tensor_sub(cmo[:], t1[:], t2[:])
                        smo = setupp.tile([P, hFD], F32, tag=f"smo{m % 4}")
                        nc.vector.tensor_add(smo[:], t3[:], t4[:])
                        nc.gpsimd.tensor_mul(frow(2 * m - 1), cmo[:], ssin[:])
                        nc.gpsimd.tensor_mul(frow(2 * m), smo[:], ssin[:])
                        cm_prev, sm_prev = cmo[:], smo[:]

            # ---- per-chunk pipeline ---------------------------------------
            with (
                tc.tile_pool(name="pst", bufs=2, space="PSUM") as pstp,
                tc.tile_pool(name="psT", bufs=3, space="PSUM") as psTp,
                tc.tile_pool(name="psP", bufs=3, space="PSUM") as psPp,
                tc.tile_pool(name="phi", bufs=3) as phip,
                tc.tile_pool(name="trigc", bufs=4) as trigcp,
                tc.tile_pool(name="gfac", bufs=2) as gfacp,
                tc.tile_pool(name="outp", bufs=2) as outp,
            ):
                assert GROUP % 2 == 0
                for h in range(NH):
                    build_features(h)
                    for g in range(h * CH // GROUP, (h + 1) * CH // GROUP):
                        # factor pairs, split-major: [128, {G1set,G2set}, GROUP*RANK]
                        gp = gfacp.tile([P, 2, GROUP, RANK], F32)
                        for cc2 in range(GROUP // 2):
                            # two chunks share one transpose PSUM tile + copy
                            pht = pstp.tile([MM_K, 2, P], F32)
                            phs = phip.tile([MM_K, 2, P], MM_DTYPE)
                            for e in range(2):
                                c = g * GROUP + cc2 * 2 + e
                                fin = feat[:, :, c::C]        # [128, 62, 2]
                                nc.tensor.transpose(pht[:, e, :], fin, idt[:])
                            nc.scalar.copy(phs[:], pht[:])    # cast to f32r
                            for e in range(2):
                                cc = cc2 * 2 + e
                                pT = psTp.tile([P, 2 * RANK], F32)
                                nc.tensor.matmul(pT[:], phs[:, e, :], rA[:],
                                                 start=True, stop=True)
                                pP = psPp.tile([P, 2 * RANK], F32)
                                nc.tensor.matmul(pP[:], phs[:, e, :], rB[:],
                                                 start=True, stop=True)
                                tc_sb = trigcp.tile([P, 2 * RANK], F32)
                                nc.scalar.copy(tc_sb[:], pT[:])
                                nc.vector.tensor_mul(gp[:, :, cc, :], tc_sb[:],
                                                     pP[:])
                        ot = outp.tile([P, GROUP * RANK], F32)
                        otv = ot[:].rearrange("p (c r) -> p c r", r=RANK)
                        nc.gpsimd.tensor_mul(otv, gp[:, 0, :, :], gp[:, 1, :, :])
                        nc.sync.dma_start(
                            out_v[:, g * GROUP:(g + 1) * GROUP, :], ot[:])

    _split_sync_waits(nc)
    return nc


_CACHE = {}


def _install_ntff_shim():
    """Provide antenv.axon_hooks (absent in this image) so that
    run_bass_kernel_spmd(trace=True) can NTFF-profile via the axon .so."""
    import contextlib
    import ctypes
    import sys
    import types

    if "antenv.axon_hooks" in sys.modules:
        return
    so_path = "/opt/axon/libaxon_pjrt.so"
    lib = ctypes.CDLL(so_path)
    lib.axon_start_nrt_profile.argtypes = [
        ctypes.POINTER(ctypes.c_int64), ctypes.c_size_t]
    lib.axon_start_nrt_profile.restype = ctypes.c_int64
    lib.axon_stop_nrt_profile.argtypes = [ctypes.c_char_p]
    lib.axon_stop_nrt_profile.restype = ctypes.c_int64

    @contextlib.contextmanager
    def _hook(output_dir, device_ids):
        import jax
        jax.devices()
        if device_ids:
            ids = (ctypes.c_int64 * len(device_ids))(*device_ids)
            rc = lib.axon_start_nrt_profile(ids, len(device_ids))
        else:
            rc = lib.axon_start_nrt_profile(None, 0)
        if rc != 0:
            raise RuntimeError(f"axon_start_nrt_profile rc={rc}")
        try:
            yield
        finally:
            n = lib.axon_stop_nrt_profile(str(output_dir).encode())
            print(f"ntff profile: {n} file(s) written to {output_dir}")

    mod = types.ModuleType("antenv.axon_hooks")
    mod.get_axon_ntff_profile_hook = lambda: _hook
    mod.set_axon_ntff_profile_hook = lambda h: None
    sys.modules["antenv.axon_hooks"] = mod


def kernel(coordinates, rand_i, rand_j):
    assert coordinates.shape == (N, 6)
    rhsA, rhsB = _build_rhs(np.asarray(rand_i), np.asarray(rand_j))
    identity = np.eye(P, dtype=np.float32)

    if "nc" not in _CACHE:
        _CACHE["nc"] = _build_nc()
    nc = _CACHE["nc"]

    coords = np.ascontiguousarray(coordinates, dtype=np.float32)
    in_maps = []
    for i in range(N_CORES):
        in_maps.append({
            "coords": coords[i * NPC:(i + 1) * NPC],
            "rhsA": rhsA,
            "rhsB": rhsB,
            "ident": identity,
        })
    import os
    trace = bool(os.environ.get("KERNEL_TRACE"))
    if trace:
        _install_ntff_shim()
    res = run_bass_kernel_spmd(nc, in_maps, core_ids=list(range(N_CORES)),
                               trace=trace)
    if trace:
        _CACHE["last_exec_time_ns"] = res.exec_time_ns
        _CACHE["last_profile"] = res
    out = np.concatenate([res.results[i]["out"] for i in range(N_CORES)], axis=0)
    return out.astype(np.float32)


# revision 15
# speedup vs baseline: 1.2103x; 1.1067x over previous
"""Trainium2 Bass kernel for random-tensor-product spherical harmonics.

Math: for each 6-D coordinate row, split into two 3-vectors o1, o2. For each,
real spherical harmonics Y_lm up to l=20 (K=441), then
out[n, r] = Y1[n, rand_i[r]] * Y2[n, rand_j[r]]  (RANK=256).

Factorization used on-device, per sphere (unit vector u,v,w; s=sin(theta),
cos(phi)=u/s, sin(phi)=v/s):
  Y_lm = [s^(m mod 2) * trig_m(phi)] * [poly_{l,m}(w)]
where trig_m = cos(m phi) (m>=0) or sin(|m| phi) (m<0), and
poly_{l,m}(w) = norm * (1-w^2)^floor(|m|/2) * (P_l^|m|(w)/s^|m|) is a
polynomial of degree <= 20 in w, evaluated in the Chebyshev basis T_k(w).

Device features (62 rows, computed point-major with DVE/GPSIMD/ACT):
  row 0: ones; rows 2m-1 / 2m (m=1..20): s^(m mod 2)-folded cos(m phi) /
  sin(m phi) from the complex-power recurrence; rows 41+k: T_k(w).
Per 128-point chunk: PE-transpose features -> [124,128] lhsT (both spheres),
two matmuls against host-built coefficient matrices give
  [TRIG1 | POLY1] and [TRIG2 | POLY2] in PSUM; then
  out = (TRIG1*POLY1) * (TRIG2*POLY2) via DVE (PSUM-legal muls) + GPSIMD.

Sharding: data-parallel over points across 8 NeuronCores; rand_i/rand_j are
folded into the replicated coefficient matrices on the host.
"""
import math

import numpy as np

import concourse.bass as bass
import concourse.tile as tile
from concourse import mybir
from concourse.bass_utils import run_bass_kernel_spmd

F32 = mybir.dt.float32
F32R = mybir.dt.float32r
AF = mybir.ActivationFunctionType
OP = mybir.AluOpType

N_CORES = 8
N = 262144
NPC = N // N_CORES          # 32768 points per core
P = 128                     # partitions
C = NPC // P                # 256 chunks (free columns) per core
RANK = 256
MAX_DEGREE = 20
NTRIG = 41                  # ones + (cos,sin) x m=1..20
NPOLY = 21                  # T_0..T_20
NFEAT = NTRIG + NPOLY       # 62
MM_K = 2 * NFEAT            # 124 matmul contract rows (feat x sphere)
MM_DTYPE = F32R             # fp32r: 1 cy/row at N=512; ~1e-4 rel error

GROUP = 4                   # chunks per output group (PSUM/DVE batching)

_MAX_WAITS = 1


def _split_sync_waits(nc):
    """walrus in this container rejects >1 sync wait per instruction; hoist
    excess waits onto InstNoOps inserted before the instruction."""
    counter = [0]

    def fresh_nop(engine, waits):
        counter[0] += 1
        nop = mybir.InstNoOp(name=f"Wsplit-{counter[0]}", ins=[], outs=[])
        nop.engine = engine
        nop.sync_info = mybir.SyncInfo(on_wait=list(waits), on_update=[])
        return nop

    for f in nc.m.functions:
        for bb in f.blocks:
            insts = bb.instructions
            new = []
            changed = False
            for inst in insts:
                si = getattr(inst, "sync_info", None)
                if si is not None and si.on_wait and len(si.on_wait) > _MAX_WAITS:
                    waits = list(si.on_wait)
                    rest, keep = waits[:-_MAX_WAITS], waits[-_MAX_WAITS:]
                    while rest:
                        new.append(fresh_nop(inst.engine, rest[:_MAX_WAITS]))
                        rest = rest[_MAX_WAITS:]
                    si.on_wait = keep
                    inst.sync_info = si
                    changed = True
                new.append(inst)
            if changed:
                bb.instructions = new


# ---------------------------------------------------------------------------
# Host-side coefficient construction
# ---------------------------------------------------------------------------

def _legendre_q(l, m, x):
    """Q_{l,m}(x) = P_l^m(x)/s^m (reference recurrence with Condon-Shortley),
    evaluated in float64. x: ndarray."""
    q_prev = np.ones_like(x)            # Q[m',m'] chain
    for mp in range(1, m + 1):
        q_prev = -(2 * mp - 1) * q_prev
    if l == m:
        return q_prev * np.ones_like(x)
    q_mm = q_prev * np.ones_like(x)
    q_next = (2 * m + 1) * x * q_mm     # Q[m+1,m]
    if l == m + 1:
        return q_next
    qa, qb = q_mm, q_next
    for ll in range(m + 2, l + 1):
        qc = ((2 * ll - 1) * x * qb - (ll + m - 1) * qa) / (ll - m)
        qa, qb = qb, qc
    return qb


def _poly_cheb_coeffs(l, m_abs, m_is_zero):
    """Chebyshev coefficients (degree<=20) of
    norm_eff * (1-w^2)^floor(m/2) * Q_{l,m}(w)."""
    norm = math.sqrt((2 * l + 1) / (4.0 * math.pi)
                     * math.factorial(l - m_abs) / math.factorial(l + m_abs))
    if not m_is_zero:
        norm *= math.sqrt(2.0)
    t = np.arange(21, dtype=np.float64)
    w = np.cos(np.pi * (t + 0.5) / 21.0)      # Chebyshev nodes
    vals = norm * (1.0 - w * w) ** (m_abs // 2) * _legendre_q(l, m_abs, w)
    return np.polynomial.chebyshev.chebfit(w, vals, MAX_DEGREE)


def _build_rhs(rand_i, rand_j):
    """Two [MM_K, 512] f32 matrices.

    rhsT: cols 0:256 one-hot trig gather for sphere 1 (rows 2*f), cols
    256:512 for sphere 2 (rows 2*f+1).  rhsP: same column split with the
    poly Chebyshev coefficients.  So MM outputs are [TRIG1|TRIG2] and
    [POLY1|POLY2]."""
    rhsT = np.zeros((MM_K, 2 * RANK), dtype=np.float64)
    rhsP = np.zeros((MM_K, 2 * RANK), dtype=np.float64)
    for sphere, rand_idx in ((0, rand_i), (1, rand_j)):
        for r in range(RANK):
            idx = int(rand_idx[r])
            l = int(math.isqrt(idx))
            m = idx - l * l - l
            m_abs = abs(m)
            if m == 0:
                trig_row = 0
            elif m > 0:
                trig_row = 2 * m - 1          # cos row
            else:
                trig_row = 2 * m_abs          # sin row
            rhsT[2 * trig_row + sphere, sphere * RANK + r] = 1.0
            coeffs = _poly_cheb_coeffs(l, m_abs, m == 0)
            for k in range(MAX_DEGREE + 1):
                rhsP[2 * (NTRIG + k) + sphere, sphere * RANK + r] = coeffs[k]
    return rhsT.astype(np.float32), rhsP.astype(np.float32)


# ---------------------------------------------------------------------------
# Device kernel
# ---------------------------------------------------------------------------

def _build_nc():
    nc = bass.Bass("TRN2", target_bir_lowering=False, debug=False)
    coords = nc.declare_dram_parameter("coords", [NPC, 6], F32, isOutput=False)
    rhsA = nc.declare_dram_parameter("rhsA", [MM_K, 2 * RANK], MM_DTYPE,
                                     isOutput=False)
    rhsB = nc.declare_dram_parameter("rhsB", [MM_K, 2 * RANK], MM_DTYPE,
                                     isOutput=False)
    ident = nc.declare_dram_parameter("ident", [P, P], F32, isOutput=False)
    out = nc.declare_dram_parameter("out", [NPC, RANK], F32, isOutput=True)

    # DRAM views
    coords_v = coords.rearrange("(p c) d -> p (c d)", p=P)       # [128, 1536]
    out_v = out.rearrange("(p c) r -> p c r", p=P)               # [128, 256, 256]

    with tile.TileContext(nc) as tc:
        with (
            tc.tile_pool(name="const", bufs=1) as constp,
            tc.tile_pool(name="feat", bufs=1) as featp,
            tc.tile_pool(name="setup", bufs=1) as setupp,
        ):
            # ---- constants -------------------------------------------------
            rA = constp.tile([MM_K, 2 * RANK], MM_DTYPE)
            nc.sync.dma_start(rA[:], rhsA[:])
            rB = constp.tile([MM_K, 2 * RANK], MM_DTYPE)
            nc.sync.dma_start(rB[:], rhsB[:])
            idt = constp.tile([P, P], F32)
            nc.sync.dma_start(idt[:], ident[:])

            # ---- coordinates + feature region -----------------------------
            raw = setupp.tile([P, C * 6], F32)
            nc.sync.dma_start(raw[:], coords_v[:])
            raw3 = raw[:].rearrange("p (c d) -> p c d", d=6)
            FD = 2 * C          # both-sphere free dim: sphere1 | sphere2
            feat = featp.tile([P, NFEAT, FD], F32)

            NH = 2              # feature halves (chunk ranges) for overlap
            CH = C // NH

            def build_features(h):
                """Features for chunks [h*CH, (h+1)*CH) of both spheres.
                Per-op views are [128, 2, CH] (cols {c}+{C+c})."""
                lo = h * CH

                def half(t):
                    return t[:].rearrange("p (s c) -> p s c", s=2)[:, :, lo:lo + CH]

                def frow(j):
                    return feat[:, j, :].rearrange("p (s c) -> p s c", s=2)[:, :, lo:lo + CH]

                hFD = 2 * CH
                xb = setupp.tile([P, hFD], F32, tag="xb")
                yb = setupp.tile([P, hFD], F32, tag="yb")
                zb = setupp.tile([P, hFD], F32, tag="zb")
                for s in range(2):
                    nc.scalar.copy(xb[:, s * CH:(s + 1) * CH],
                                   raw3[:, lo:lo + CH, 3 * s])
                    nc.scalar.copy(yb[:, s * CH:(s + 1) * CH],
                                   raw3[:, lo:lo + CH, 3 * s + 1])
                    nc.scalar.copy(zb[:, s * CH:(s + 1) * CH],
                                   raw3[:, lo:lo + CH, 3 * s + 2])
                x2 = setupp.tile([P, hFD], F32, tag="x2")
                nc.vector.tensor_mul(x2[:], xb[:], xb[:])
                y2 = setupp.tile([P, hFD], F32, tag="y2")
                nc.vector.tensor_mul(y2[:], yb[:], yb[:])
                rho2 = setupp.tile([P, hFD], F32, tag="rho2")
                nc.vector.tensor_add(rho2[:], x2[:], y2[:])
                z2 = setupp.tile([P, hFD], F32, tag="z2")
                nc.vector.tensor_mul(z2[:], zb[:], zb[:])
                r2 = setupp.tile([P, hFD], F32, tag="r2")
                nc.vector.tensor_add(r2[:], rho2[:], z2[:])
                rho2g = setupp.tile([P, hFD], F32, tag="rho2g")
                nc.vector.tensor_scalar_max(rho2g[:], rho2[:], 1e-30)
                rr = setupp.tile([P, hFD], F32, tag="rr")
                nc.scalar.activation(rr[:], r2[:], AF.Sqrt)
                rho = setupp.tile([P, hFD], F32, tag="rho")
                nc.scalar.activation(rho[:], rho2g[:], AF.Sqrt)
                rinv = setupp.tile([P, hFD], F32, tag="rinv")
                nc.vector.reciprocal(rinv[:], rr[:])
                rhoinv = setupp.tile([P, hFD], F32, tag="rhoinv")
                nc.vector.reciprocal(rhoinv[:], rho[:])
                w = setupp.tile([P, hFD], F32, tag="w")
                nc.vector.tensor_mul(w[:], zb[:], rinv[:])
                cphi = setupp.tile([P, hFD], F32, tag="cphi")
                nc.vector.tensor_mul(cphi[:], xb[:], rhoinv[:])
                sphi = setupp.tile([P, hFD], F32, tag="sphi")
                nc.vector.tensor_mul(sphi[:], yb[:], rhoinv[:])
                ssin = setupp.tile([P, hFD], F32, tag="ssin")
                nc.vector.tensor_mul(ssin[:], rho[:], rinv[:])

                nc.gpsimd.memset(frow(0), 1.0)            # trig m=0
                nc.gpsimd.memset(frow(NTRIG), 1.0)        # T_0

                # Chebyshev chain on GPSIMD
                w2 = setupp.tile([P, hFD], F32, tag="w2")
                nc.vector.tensor_add(w2[:], w[:], w[:])
                nc.gpsimd.tensor_copy(frow(NTRIG + 1), w[:])
                for k in range(2, MAX_DEGREE + 1):
                    tkt = setupp.tile([P, hFD], F32, tag="chebt")
                    nc.gpsimd.tensor_mul(tkt[:], w2[:], frow(NTRIG + k - 1))
                    nc.gpsimd.tensor_sub(frow(NTRIG + k), tkt[:],
                                         frow(NTRIG + k - 2))

                # trig features via four Chebyshev chains in 2*cos(2phi):
                #   cos(m phi) and s^(m mod 2)-folded rows directly, no folds.
                # rows: m-cos -> 2m-1, m-sin -> 2m.
                cp2 = setupp.tile([P, hFD], F32, tag="cp2")
                nc.vector.tensor_mul(cp2[:], cphi[:], cphi[:])
                c2d = setupp.tile([P, hFD], F32, tag="c2d")   # 2*cos(2phi)
                nc.vector.tensor_scalar(c2d[:], cp2[:], 4.0, 2.0,
                                        OP.mult, OP.subtract)
                nc.vector.tensor_scalar(frow(3), cp2[:], 2.0, 1.0,
                                        OP.mult, OP.subtract)   # cos2phi
                tq = setupp.tile([P, hFD], F32, tag="tq")
                nc.vector.tensor_mul(tq[:], cphi[:], sphi[:])
                nc.vector.tensor_add(frow(4), tq[:], tq[:])     # sin2phi
                nc.vector.tensor_mul(frow(1), ssin[:], cphi[:])  # s cos phi
                nc.vector.tensor_mul(frow(2), ssin[:], sphi[:])  # s sin phi

                # even-cos: e_j = cos(2j phi), rows 0,3,7,...  (j=0 -> ones row)
                def row_ec(j):
                    return 0 if j == 0 else 4 * j - 1
                for j in range(2, 11):
                    tt = setupp.tile([P, hFD], F32, tag=f"ch{j % 2}")
                    nc.vector.tensor_mul(tt[:], c2d[:], frow(row_ec(j - 1)))
                    nc.vector.tensor_sub(frow(row_ec(j)), tt[:],
                                         frow(row_ec(j - 2)))
                # even-sin: f_j = sin(2j phi), rows 4j; f_0 = 0
                for j in range(2, 11):
                    tt = setupp.tile([P, hFD], F32, tag=f"cs{j % 2}")
                    nc.vector.tensor_mul(tt[:], c2d[:], frow(4 * (j - 1)))
                    if j == 2:
                        nc.vector.tensor_copy(frow(8), tt[:])
                    else:
                        nc.vector.tensor_sub(frow(4 * j), tt[:],
                                             frow(4 * (j - 2)))
                # odd-cos: o_j = s cos((2j+1) phi), rows 4j+1; o_{-1} = o_0
                for j in range(1, 10):
                    tt = setupp.tile([P, hFD], F32, tag=f"co{j % 2}")
                    nc.vector.tensor_mul(tt[:], c2d[:], frow(4 * (j - 1) + 1))
                    nc.vector.tensor_sub(frow(4 * j + 1), tt[:],
                                         frow(4 * (j - 2) + 1 if j >= 2 else 1))
                # odd-sin: q_j = s sin((2j+1) phi), rows 4j+2; q_{-1} = -q_0
                for j in range(1, 10):
                    tt = setupp.tile([P, hFD], F32, tag=f"cq{j % 2}")
                    nc.vector.tensor_mul(tt[:], c2d[:], frow(4 * (j - 1) + 2))
                    if j == 1:
                        nc.vector.tensor_add(frow(6), tt[:], frow(2))
                    else:
                        nc.vector.tensor_sub(frow(4 * j + 2), tt[:],
                                             frow(4 * (j - 2) + 2))

            # ---- per-chunk pipeline ---------------------------------------
            with (
                tc.tile_pool(name="pst", bufs=2, space="PSUM") as pstp,
                tc.tile_pool(name="psT", bufs=2, space="PSUM") as psTp,
                tc.tile_pool(name="psP", bufs=2, space="PSUM") as psPp,
                tc.tile_pool(name="phi", bufs=3) as phip,
                tc.tile_pool(name="trigc", bufs=4) as trigcp,
                tc.tile_pool(name="gfac", bufs=2) as gfacp,
                tc.tile_pool(name="outp", bufs=2) as outp,
            ):
                assert GROUP % 2 == 0
                for h in range(NH):
                    build_features(h)
                    for g in range(h * CH // GROUP, (h + 1) * CH // GROUP):
                        # factor pairs, split-major: [128, {G1set,G2set}, GROUP*RANK]
                        gp = gfacp.tile([P, 2, GROUP, RANK], F32)
                        for cc2 in range(GROUP // 2):
                            # two chunks share one transpose PSUM tile + copy
                            pht = pstp.tile([MM_K, 2, P], F32)
                            phs = phip.tile([MM_K, 2, P], MM_DTYPE)
                            for e in range(2):
                                c = g * GROUP + cc2 * 2 + e
                                fin = feat[:, :, c::C]        # [128, 62, 2]
                                nc.tensor.transpose(pht[:, e, :], fin, idt[:])
                            nc.scalar.copy(phs[:], pht[:])    # cast to f32r
                            pP2 = psPp.tile([P, 2, 2 * RANK], F32)  # 2 banks
                            tc2 = trigcp.tile([P, 2, 2 * RANK], F32)
                            for e in range(2):
                                pT = psTp.tile([P, 2 * RANK], F32)
                                nc.tensor.matmul(pT[:], phs[:, e, :], rA[:],
                                                 start=True, stop=True)
                                nc.tensor.matmul(pP2[:, e, :], phs[:, e, :],
                                                 rB[:], start=True, stop=True)
                                nc.scalar.copy(tc2[:, e, :], pT[:])
                            # one pair-mul for both chunks:
                            # [p, sphere-set, chunk, r]
                            tc2v = tc2[:].rearrange("p c (s r) -> p s c r", s=2)
                            pP2v = pP2[:].rearrange("p c (s r) -> p s c r", s=2)
                            nc.vector.tensor_mul(
                                gp[:, :, cc2 * 2:cc2 * 2 + 2, :], tc2v, pP2v)
                        ot = outp.tile([P, GROUP * RANK], F32)
                        otv = ot[:].rearrange("p (c r) -> p c r", r=RANK)
                        nc.gpsimd.tensor_mul(otv, gp[:, 0, :, :], gp[:, 1, :, :])
                        nc.sync.dma_start(
                            out_v[:, g * GROUP:(g + 1) * GROUP, :], ot[:])

    _split_sync_waits(nc)
    return nc


_CACHE = {}


def _install_ntff_shim():
    """Provide antenv.axon_hooks (absent in this image) so that
    run_bass_kernel_spmd(trace=True) can NTFF-profile via the axon .so."""
    import contextlib
    import ctypes
    import sys
    import types

    if "antenv.axon_hooks" in sys.modules:
        return
    so_path = "/opt/axon/libaxon_pjrt.so"
    lib = ctypes.CDLL(so_path)
    lib.axon_start_nrt_profile.argtypes = [
        ctypes.POINTER(ctypes.c_int64), ctypes.c_size_t]
    lib.axon_start_nrt_profile.restype = ctypes.c_int64
    lib.axon_stop_nrt_profile.argtypes = [ctypes.c_char_p]
    lib.axon_stop_nrt_profile.restype = ctypes.c_int64

    @contextlib.contextmanager
    def _hook(output_dir, device_ids):
        import jax
        jax.devices()
        if device_ids:
            ids = (ctypes.c_int64 * len(device_ids))(*device_ids)
            rc = lib.axon_start_nrt_profile(ids, len(device_ids))
        else:
            rc = lib.axon_start_nrt_profile(None, 0)
        if rc != 0:
            raise RuntimeError(f"axon_start_nrt_profile rc={rc}")
        try:
            yield
        finally:
            n = lib.axon_stop_nrt_profile(str(output_dir).encode())
            print(f"ntff profile: {n} file(s) written to {output_dir}")

    mod = types.ModuleType("antenv.axon_hooks")
    mod.get_axon_ntff_profile_hook = lambda: _hook
    mod.set_axon_ntff_profile_hook = lambda h: None
    sys.modules["antenv.axon_hooks"] = mod


def kernel(coordinates, rand_i, rand_j):
    assert coordinates.shape == (N, 6)
    rhsA, rhsB = _build_rhs(np.asarray(rand_i), np.asarray(rand_j))
    identity = np.eye(P, dtype=np.float32)

    if "nc" not in _CACHE:
        _CACHE["nc"] = _build_nc()
    nc = _CACHE["nc"]

    coords = np.ascontiguousarray(coordinates, dtype=np.float32)
    in_maps = []
    for i in range(N_CORES):
        in_maps.append({
            "coords": coords[i * NPC:(i + 1) * NPC],
            "rhsA": rhsA,
            "rhsB": rhsB,
            "ident": identity,
        })
    import os
    trace = bool(os.environ.get("KERNEL_TRACE"))
    if trace:
        _install_ntff_shim()
    res = run_bass_kernel_spmd(nc, in_maps, core_ids=list(range(N_CORES)),
                               trace=trace)
    if trace:
        _CACHE["last_exec_time_ns"] = res.exec_time_ns
        _CACHE["last_profile"] = res
    out = np.concatenate([res.results[i]["out"] for i in range(N_CORES)], axis=0)
    return out.astype(np.float32)


# revision 17
# speedup vs baseline: 1.2156x; 1.0044x over previous
"""Trainium2 Bass kernel for random-tensor-product spherical harmonics.

Math: for each 6-D coordinate row, split into two 3-vectors o1, o2. For each,
real spherical harmonics Y_lm up to l=20 (K=441), then
out[n, r] = Y1[n, rand_i[r]] * Y2[n, rand_j[r]]  (RANK=256).

Factorization used on-device, per sphere (unit vector u,v,w; s=sin(theta),
cos(phi)=u/s, sin(phi)=v/s):
  Y_lm = [s^(m mod 2) * trig_m(phi)] * [poly_{l,m}(w)]
where trig_m = cos(m phi) (m>=0) or sin(|m| phi) (m<0), and
poly_{l,m}(w) = norm * (1-w^2)^floor(|m|/2) * (P_l^|m|(w)/s^|m|) is a
polynomial of degree <= 20 in w, evaluated in the Chebyshev basis T_k(w).

Device features (62 rows, computed point-major with DVE/GPSIMD/ACT):
  row 0: ones; rows 2m-1 / 2m (m=1..20): s^(m mod 2)-folded cos(m phi) /
  sin(m phi) from the complex-power recurrence; rows 41+k: T_k(w).
Per 128-point chunk: PE-transpose features -> [124,128] lhsT (both spheres),
two matmuls against host-built coefficient matrices give
  [TRIG1 | POLY1] and [TRIG2 | POLY2] in PSUM; then
  out = (TRIG1*POLY1) * (TRIG2*POLY2) via DVE (PSUM-legal muls) + GPSIMD.

Sharding: data-parallel over points across 8 NeuronCores; rand_i/rand_j are
folded into the replicated coefficient matrices on the host.
"""
import math

import numpy as np

import concourse.bass as bass
import concourse.tile as tile
from concourse import mybir
from concourse.bass_utils import run_bass_kernel_spmd

F32 = mybir.dt.float32
F32R = mybir.dt.float32r
AF = mybir.ActivationFunctionType
OP = mybir.AluOpType

N_CORES = 8
N = 262144
NPC = N // N_CORES          # 32768 points per core
P = 128                     # partitions
C = NPC // P                # 256 chunks (free columns) per core
RANK = 256
MAX_DEGREE = 20
NTRIG = 41                  # ones + (cos,sin) x m=1..20
NPOLY = 21                  # T_0..T_20
NFEAT = NTRIG + NPOLY       # 62
MM_K = 2 * NFEAT            # 124 matmul contract rows (feat x sphere)
MM_DTYPE = F32R             # fp32r: 1 cy/row at N=512; ~1e-4 rel error

GROUP = 4                   # chunks per output group (PSUM/DVE batching)

_MAX_WAITS = 1


def _split_sync_waits(nc):
    """walrus in this container rejects >1 sync wait per instruction; hoist
    excess waits onto InstNoOps inserted before the instruction."""
    counter = [0]

    def fresh_nop(engine, waits):
        counter[0] += 1
        nop = mybir.InstNoOp(name=f"Wsplit-{counter[0]}", ins=[], outs=[])
        nop.engine = engine
        nop.sync_info = mybir.SyncInfo(on_wait=list(waits), on_update=[])
        return nop

    for f in nc.m.functions:
        for bb in f.blocks:
            insts = bb.instructions
            new = []
            changed = False
            for inst in insts:
                si = getattr(inst, "sync_info", None)
                if si is not None and si.on_wait and len(si.on_wait) > _MAX_WAITS:
                    waits = list(si.on_wait)
                    rest, keep = waits[:-_MAX_WAITS], waits[-_MAX_WAITS:]
                    while rest:
                        new.append(fresh_nop(inst.engine, rest[:_MAX_WAITS]))
                        rest = rest[_MAX_WAITS:]
                    si.on_wait = keep
                    inst.sync_info = si
                    changed = True
                new.append(inst)
            if changed:
                bb.instructions = new


# ---------------------------------------------------------------------------
# Host-side coefficient construction
# ---------------------------------------------------------------------------

def _legendre_q(l, m, x):
    """Q_{l,m}(x) = P_l^m(x)/s^m (reference recurrence with Condon-Shortley),
    evaluated in float64. x: ndarray."""
    q_prev = np.ones_like(x)            # Q[m',m'] chain
    for mp in range(1, m + 1):
        q_prev = -(2 * mp - 1) * q_prev
    if l == m:
        return q_prev * np.ones_like(x)
    q_mm = q_prev * np.ones_like(x)
    q_next = (2 * m + 1) * x * q_mm     # Q[m+1,m]
    if l == m + 1:
        return q_next
    qa, qb = q_mm, q_next
    for ll in range(m + 2, l + 1):
        qc = ((2 * ll - 1) * x * qb - (ll + m - 1) * qa) / (ll - m)
        qa, qb = qb, qc
    return qb


def _poly_cheb_coeffs(l, m_abs, m_is_zero):
    """Chebyshev coefficients (degree<=20) of
    norm_eff * (1-w^2)^floor(m/2) * Q_{l,m}(w)."""
    norm = math.sqrt((2 * l + 1) / (4.0 * math.pi)
                     * math.factorial(l - m_abs) / math.factorial(l + m_abs))
    if not m_is_zero:
        norm *= math.sqrt(2.0)
    t = np.arange(21, dtype=np.float64)
    w = np.cos(np.pi * (t + 0.5) / 21.0)      # Chebyshev nodes
    vals = norm * (1.0 - w * w) ** (m_abs // 2) * _legendre_q(l, m_abs, w)
    return np.polynomial.chebyshev.chebfit(w, vals, MAX_DEGREE)


def _build_rhs(rand_i, rand_j):
    """Two [MM_K, 512] f32 matrices.

    rhsT: cols 0:256 one-hot trig gather for sphere 1 (rows 2*f), cols
    256:512 for sphere 2 (rows 2*f+1).  rhsP: same column split with the
    poly Chebyshev coefficients.  So MM outputs are [TRIG1|TRIG2] and
    [POLY1|POLY2]."""
    rhsT = np.zeros((MM_K, 2 * RANK), dtype=np.float64)
    rhsP = np.zeros((MM_K, 2 * RANK), dtype=np.float64)
    for sphere, rand_idx in ((0, rand_i), (1, rand_j)):
        for r in range(RANK):
            idx = int(rand_idx[r])
            l = int(math.isqrt(idx))
            m = idx - l * l - l
            m_abs = abs(m)
            if m == 0:
                trig_row = 0
            elif m > 0:
                trig_row = 2 * m - 1          # cos row
            else:
                trig_row = 2 * m_abs          # sin row
            rhsT[2 * trig_row + sphere, sphere * RANK + r] = 1.0
            coeffs = _poly_cheb_coeffs(l, m_abs, m == 0)
            for k in range(MAX_DEGREE + 1):
                rhsP[2 * (NTRIG + k) + sphere, sphere * RANK + r] = coeffs[k]
    return rhsT.astype(np.float32), rhsP.astype(np.float32)


# ---------------------------------------------------------------------------
# Device kernel
# ---------------------------------------------------------------------------

def _build_nc():
    nc = bass.Bass("TRN2", target_bir_lowering=False, debug=False)
    coords = nc.declare_dram_parameter("coords", [NPC, 6], F32, isOutput=False)
    rhsA = nc.declare_dram_parameter("rhsA", [MM_K, 2 * RANK], MM_DTYPE,
                                     isOutput=False)
    rhsB = nc.declare_dram_parameter("rhsB", [MM_K, 2 * RANK], MM_DTYPE,
                                     isOutput=False)
    ident = nc.declare_dram_parameter("ident", [P, P], F32, isOutput=False)
    out = nc.declare_dram_parameter("out", [NPC, RANK], F32, isOutput=True)

    # DRAM views
    coords_v = coords.rearrange("(p c) d -> p (c d)", p=P)       # [128, 1536]
    out_v = out.rearrange("(p c) r -> p c r", p=P)               # [128, 256, 256]

    with tile.TileContext(nc) as tc:
        with (
            tc.tile_pool(name="const", bufs=1) as constp,
            tc.tile_pool(name="feat", bufs=1) as featp,
            tc.tile_pool(name="setup", bufs=1) as setupp,
        ):
            # ---- constants -------------------------------------------------
            rA = constp.tile([MM_K, 2 * RANK], MM_DTYPE)
            nc.sync.dma_start(rA[:], rhsA[:])
            rB = constp.tile([MM_K, 2 * RANK], MM_DTYPE)
            nc.sync.dma_start(rB[:], rhsB[:])
            idt = constp.tile([P, P], F32)
            nc.sync.dma_start(idt[:], ident[:])

            # ---- coordinates + feature region -----------------------------
            raw = setupp.tile([P, C * 6], F32)
            nc.sync.dma_start(raw[:], coords_v[:])
            raw3 = raw[:].rearrange("p (c d) -> p c d", d=6)
            NH = 2              # feature halves (chunk ranges) for overlap
            CH = C // NH
            feats = []
            for h in range(NH):
                fh = featp.tile([P, NFEAT, 2 * CH], F32, tag=f"feat{h}",
                                name=f"feat{h}")
                feats.append(fh)

            def build_features(h):
                """Features for chunks [h*CH, (h+1)*CH) of both spheres.
                Region cols: [sphere1 c=0..CH-1 | sphere2 c=0..CH-1]."""
                lo = h * CH
                feat = feats[h]

                def frow(j):
                    return feat[:, j, :]

                hFD = 2 * CH
                xb = setupp.tile([P, hFD], F32, tag="xb")
                yb = setupp.tile([P, hFD], F32, tag="yb")
                zb = setupp.tile([P, hFD], F32, tag="zb")
                for s in range(2):
                    nc.scalar.copy(xb[:, s * CH:(s + 1) * CH],
                                   raw3[:, lo:lo + CH, 3 * s])
                    nc.scalar.copy(yb[:, s * CH:(s + 1) * CH],
                                   raw3[:, lo:lo + CH, 3 * s + 1])
                    nc.scalar.copy(zb[:, s * CH:(s + 1) * CH],
                                   raw3[:, lo:lo + CH, 3 * s + 2])
                x2 = setupp.tile([P, hFD], F32, tag="x2")
                nc.vector.tensor_mul(x2[:], xb[:], xb[:])
                y2 = setupp.tile([P, hFD], F32, tag="y2")
                nc.vector.tensor_mul(y2[:], yb[:], yb[:])
                rho2 = setupp.tile([P, hFD], F32, tag="rho2")
                nc.vector.tensor_add(rho2[:], x2[:], y2[:])
                z2 = setupp.tile([P, hFD], F32, tag="z2")
                nc.vector.tensor_mul(z2[:], zb[:], zb[:])
                r2 = setupp.tile([P, hFD], F32, tag="r2")
                nc.vector.tensor_add(r2[:], rho2[:], z2[:])
                rho2g = setupp.tile([P, hFD], F32, tag="rho2g")
                nc.vector.tensor_scalar_max(rho2g[:], rho2[:], 1e-30)
                rr = setupp.tile([P, hFD], F32, tag="rr")
                nc.scalar.activation(rr[:], r2[:], AF.Sqrt)
                rho = setupp.tile([P, hFD], F32, tag="rho")
                nc.scalar.activation(rho[:], rho2g[:], AF.Sqrt)
                rinv = setupp.tile([P, hFD], F32, tag="rinv")
                nc.vector.reciprocal(rinv[:], rr[:])
                rhoinv = setupp.tile([P, hFD], F32, tag="rhoinv")
                nc.vector.reciprocal(rhoinv[:], rho[:])
                w = setupp.tile([P, hFD], F32, tag="w")
                nc.vector.tensor_mul(w[:], zb[:], rinv[:])
                cphi = setupp.tile([P, hFD], F32, tag="cphi")
                nc.vector.tensor_mul(cphi[:], xb[:], rhoinv[:])
                sphi = setupp.tile([P, hFD], F32, tag="sphi")
                nc.vector.tensor_mul(sphi[:], yb[:], rhoinv[:])
                ssin = setupp.tile([P, hFD], F32, tag="ssin")
                nc.vector.tensor_mul(ssin[:], rho[:], rinv[:])

                nc.gpsimd.memset(frow(0), 1.0)            # trig m=0
                nc.gpsimd.memset(frow(NTRIG), 1.0)        # T_0

                # Chebyshev chain on GPSIMD
                w2 = setupp.tile([P, hFD], F32, tag="w2")
                nc.vector.tensor_add(w2[:], w[:], w[:])
                nc.gpsimd.tensor_copy(frow(NTRIG + 1), w[:])
                for k in range(2, MAX_DEGREE + 1):
                    tkt = setupp.tile([P, hFD], F32, tag="chebt")
                    nc.gpsimd.tensor_mul(tkt[:], w2[:], frow(NTRIG + k - 1))
                    nc.gpsimd.tensor_sub(frow(NTRIG + k), tkt[:],
                                         frow(NTRIG + k - 2))

                # trig features via four Chebyshev chains in 2*cos(2phi):
                #   cos(m phi) and s^(m mod 2)-folded rows directly, no folds.
                # rows: m-cos -> 2m-1, m-sin -> 2m.
                cp2 = setupp.tile([P, hFD], F32, tag="cp2")
                nc.vector.tensor_mul(cp2[:], cphi[:], cphi[:])
                c2d = setupp.tile([P, hFD], F32, tag="c2d")   # 2*cos(2phi)
                nc.vector.tensor_scalar(c2d[:], cp2[:], 4.0, 2.0,
                                        OP.mult, OP.subtract)
                nc.vector.tensor_scalar(frow(3), cp2[:], 2.0, 1.0,
                                        OP.mult, OP.subtract)   # cos2phi
                tq = setupp.tile([P, hFD], F32, tag="tq")
                nc.vector.tensor_mul(tq[:], cphi[:], sphi[:])
                nc.vector.tensor_add(frow(4), tq[:], tq[:])     # sin2phi
                nc.vector.tensor_mul(frow(1), ssin[:], cphi[:])  # s cos phi
                nc.vector.tensor_mul(frow(2), ssin[:], sphi[:])  # s sin phi

                # even-cos: e_j = cos(2j phi), rows 0,3,7,...  (j=0 -> ones row)
                def row_ec(j):
                    return 0 if j == 0 else 4 * j - 1
                for j in range(2, 11):
                    tt = setupp.tile([P, hFD], F32, tag=f"ch{j % 2}")
                    nc.vector.tensor_mul(tt[:], c2d[:], frow(row_ec(j - 1)))
                    nc.vector.tensor_sub(frow(row_ec(j)), tt[:],
                                         frow(row_ec(j - 2)))
                # even-sin: f_j = sin(2j phi), rows 4j; f_0 = 0
                for j in range(2, 11):
                    tt = setupp.tile([P, hFD], F32, tag=f"cs{j % 2}")
                    nc.vector.tensor_mul(tt[:], c2d[:], frow(4 * (j - 1)))
                    if j == 2:
                        nc.vector.tensor_copy(frow(8), tt[:])
                    else:
                        nc.vector.tensor_sub(frow(4 * j), tt[:],
                                             frow(4 * (j - 2)))
                # odd-cos: o_j = s cos((2j+1) phi), rows 4j+1; o_{-1} = o_0
                for j in range(1, 10):
                    tt = setupp.tile([P, hFD], F32, tag=f"co{j % 2}")
                    nc.vector.tensor_mul(tt[:], c2d[:], frow(4 * (j - 1) + 1))
                    nc.vector.tensor_sub(frow(4 * j + 1), tt[:],
                                         frow(4 * (j - 2) + 1 if j >= 2 else 1))
                # odd-sin: q_j = s sin((2j+1) phi), rows 4j+2; q_{-1} = -q_0
                for j in range(1, 10):
                    tt = setupp.tile([P, hFD], F32, tag=f"cq{j % 2}")
                    nc.vector.tensor_mul(tt[:], c2d[:], frow(4 * (j - 1) + 2))
                    if j == 1:
                        nc.vector.tensor_add(frow(6), tt[:], frow(2))
                    else:
                        nc.vector.tensor_sub(frow(4 * j + 2), tt[:],
                                             frow(4 * (j - 2) + 2))

            # ---- per-chunk pipeline ---------------------------------------
            with (
                tc.tile_pool(name="pst", bufs=2, space="PSUM") as pstp,
                tc.tile_pool(name="psT", bufs=2, space="PSUM") as psTp,
                tc.tile_pool(name="psP", bufs=2, space="PSUM") as psPp,
                tc.tile_pool(name="phi", bufs=3) as phip,
                tc.tile_pool(name="trigc", bufs=4) as trigcp,
                tc.tile_pool(name="gfac", bufs=2) as gfacp,
                tc.tile_pool(name="outp", bufs=2) as outp,
            ):
                assert GROUP % 2 == 0
                for h in range(NH):
                    build_features(h)
                    for g in range(h * CH // GROUP, (h + 1) * CH // GROUP):
                        # factor pairs, split-major: [128, {G1set,G2set}, GROUP*RANK]
                        gp = gfacp.tile([P, GROUP, 2, RANK], F32)
                        for cc2 in range(GROUP // 2):
                            # two chunks share one transpose PSUM tile + copy
                            pht = pstp.tile([MM_K, 2, P], F32)
                            phs = phip.tile([MM_K, 2, P], MM_DTYPE)
                            for e in range(2):
                                c = g * GROUP + cc2 * 2 + e
                                cloc = c % CH
                                fin = feats[c // CH][:, :, cloc::CH]  # [128,62,2]
                                nc.tensor.transpose(pht[:, e, :], fin, idt[:])
                            nc.scalar.copy(phs[:], pht[:])    # cast to f32r
                            pP2 = psPp.tile([P, 2, 2 * RANK], F32)  # 2 banks
                            tc2 = trigcp.tile([P, 2, 2 * RANK], F32)
                            for e in range(2):
                                pT = psTp.tile([P, 2 * RANK], F32)
                                nc.tensor.matmul(pT[:], phs[:, e, :], rA[:],
                                                 start=True, stop=True)
                                nc.tensor.matmul(pP2[:, e, :], phs[:, e, :],
                                                 rB[:], start=True, stop=True)
                                nc.scalar.copy(tc2[:, e, :], pT[:])
                            # one pair-mul for both chunks (natural order)
                            nc.vector.tensor_mul(
                                gp[:, cc2 * 2:cc2 * 2 + 2, :, :], tc2[:], pP2[:])
                        ot = outp.tile([P, GROUP * RANK], F32)
                        otv = ot[:].rearrange("p (c r) -> p c r", r=RANK)
                        nc.gpsimd.tensor_mul(otv, gp[:, :, 0, :], gp[:, :, 1, :])
                        nc.sync.dma_start(
                            out_v[:, g * GROUP:(g + 1) * GROUP, :], ot[:])

    _split_sync_waits(nc)
    return nc


_CACHE = {}


def _install_ntff_shim():
    """Provide antenv.axon_hooks (absent in this image) so that
    run_bass_kernel_spmd(trace=True) can NTFF-profile via the axon .so."""
    import contextlib
    import ctypes
    import sys
    import types

    if "antenv.axon_hooks" in sys.modules:
        return
    so_path = "/opt/axon/libaxon_pjrt.so"
    lib = ctypes.CDLL(so_path)
    lib.axon_start_nrt_profile.argtypes = [
        ctypes.POINTER(ctypes.c_int64), ctypes.c_size_t]
    lib.axon_start_nrt_profile.restype = ctypes.c_int64
    lib.axon_stop_nrt_profile.argtypes = [ctypes.c_char_p]
    lib.axon_stop_nrt_profile.restype = ctypes.c_int64

    @contextlib.contextmanager
    def _hook(output_dir, device_ids):
        import jax
        jax.devices()
        if device_ids:
            ids = (ctypes.c_int64 * len(device_ids))(*device_ids)
            rc = lib.axon_start_nrt_profile(ids, len(device_ids))
        else:
            rc = lib.axon_start_nrt_profile(None, 0)
        if rc != 0:
            raise RuntimeError(f"axon_start_nrt_profile rc={rc}")
        try:
            yield
        finally:
            n = lib.axon_stop_nrt_profile(str(output_dir).encode())
            print(f"ntff profile: {n} file(s) written to {output_dir}")

    mod = types.ModuleType("antenv.axon_hooks")
    mod.get_axon_ntff_profile_hook = lambda: _hook
    mod.set_axon_ntff_profile_hook = lambda h: None
    sys.modules["antenv.axon_hooks"] = mod


def kernel(coordinates, rand_i, rand_j):
    assert coordinates.shape == (N, 6)
    rhsA, rhsB = _build_rhs(np.asarray(rand_i), np.asarray(rand_j))
    identity = np.eye(P, dtype=np.float32)

    if "nc" not in _CACHE:
        _CACHE["nc"] = _build_nc()
    nc = _CACHE["nc"]

    coords = np.ascontiguousarray(coordinates, dtype=np.float32)
    in_maps = []
    for i in range(N_CORES):
        in_maps.append({
            "coords": coords[i * NPC:(i + 1) * NPC],
            "rhsA": rhsA,
            "rhsB": rhsB,
            "ident": identity,
        })
    import os
    trace = bool(os.environ.get("KERNEL_TRACE"))
    if trace:
        _install_ntff_shim()
    res = run_bass_kernel_spmd(nc, in_maps, core_ids=list(range(N_CORES)),
                               trace=trace)
    if trace:
        _CACHE["last_exec_time_ns"] = res.exec_time_ns
        _CACHE["last_profile"] = res
    out = np.concatenate([res.results[i]["out"] for i in range(N_CORES)], axis=0)
    return out.astype(np.float32)


# revision 21
# speedup vs baseline: 1.4013x; 1.1528x over previous
"""Trainium2 Bass kernel for random-tensor-product spherical harmonics.

Math: for each 6-D coordinate row, split into two 3-vectors o1, o2. For each,
real spherical harmonics Y_lm up to l=20 (K=441), then
out[n, r] = Y1[n, rand_i[r]] * Y2[n, rand_j[r]]  (RANK=256).

Factorization used on-device, per sphere (unit vector u,v,w; s=sin(theta),
cos(phi)=u/s, sin(phi)=v/s):
  Y_lm = [s^(m mod 2) * trig_m(phi)] * [poly_{l,m}(w)]
where trig_m = cos(m phi) (m>=0) or sin(|m| phi) (m<0), and
poly_{l,m}(w) = norm * (1-w^2)^floor(|m|/2) * (P_l^|m|(w)/s^|m|) is a
polynomial of degree <= 20 in w, evaluated in the Chebyshev basis T_k(w).

Device features (62 rows, computed point-major with DVE/GPSIMD/ACT):
  row 0: ones; rows 2m-1 / 2m (m=1..20): s^(m mod 2)-folded cos(m phi) /
  sin(m phi) from the complex-power recurrence; rows 41+k: T_k(w).
Per 128-point chunk: PE-transpose features -> [124,128] lhsT (both spheres),
two matmuls against host-built coefficient matrices give
  [TRIG1 | POLY1] and [TRIG2 | POLY2] in PSUM; then
  out = (TRIG1*POLY1) * (TRIG2*POLY2) via DVE (PSUM-legal muls) + GPSIMD.

Sharding: data-parallel over points across 8 NeuronCores; rand_i/rand_j are
folded into the replicated coefficient matrices on the host.
"""
import math

import numpy as np

import concourse.bass as bass
import concourse.tile as tile
from concourse import mybir
from concourse.bass_utils import run_bass_kernel_spmd

F32 = mybir.dt.float32
F32R = mybir.dt.float32r
AF = mybir.ActivationFunctionType
OP = mybir.AluOpType

N_CORES = 8
N = 262144
NPC = N // N_CORES          # 32768 points per core
P = 128                     # partitions
C = NPC // P                # 256 chunks (free columns) per core
RANK = 256
MAX_DEGREE = 20
NTRIG = 41                  # ones + (cos,sin) x m=1..20
NPOLY = 21                  # T_0..T_20
NFEAT = NTRIG + NPOLY       # 62
MM_K = 2 * NFEAT            # 124 matmul contract rows (feat x sphere)
MM_DTYPE = F32R             # fp32r: 1 cy/row at N=512; ~1e-4 rel error

GROUP = 4                   # chunks per output group (PSUM/DVE batching)

_MAX_WAITS = 1


def _split_sync_waits(nc):
    """walrus in this container rejects >1 sync wait per instruction; hoist
    excess waits onto InstNoOps inserted before the instruction."""
    counter = [0]

    def fresh_nop(engine, waits):
        counter[0] += 1
        nop = mybir.InstNoOp(name=f"Wsplit-{counter[0]}", ins=[], outs=[])
        nop.engine = engine
        nop.sync_info = mybir.SyncInfo(on_wait=list(waits), on_update=[])
        return nop

    for f in nc.m.functions:
        for bb in f.blocks:
            insts = bb.instructions
            new = []
            changed = False
            for inst in insts:
                si = getattr(inst, "sync_info", None)
                if si is not None and si.on_wait and len(si.on_wait) > _MAX_WAITS:
                    waits = list(si.on_wait)
                    rest, keep = waits[:-_MAX_WAITS], waits[-_MAX_WAITS:]
                    while rest:
                        new.append(fresh_nop(inst.engine, rest[:_MAX_WAITS]))
                        rest = rest[_MAX_WAITS:]
                    si.on_wait = keep
                    inst.sync_info = si
                    changed = True
                new.append(inst)
            if changed:
                bb.instructions = new


# ---------------------------------------------------------------------------
# Host-side coefficient construction
# ---------------------------------------------------------------------------

def _legendre_q(l, m, x):
    """Q_{l,m}(x) = P_l^m(x)/s^m (reference recurrence with Condon-Shortley),
    evaluated in float64. x: ndarray."""
    q_prev = np.ones_like(x)            # Q[m',m'] chain
    for mp in range(1, m + 1):
        q_prev = -(2 * mp - 1) * q_prev
    if l == m:
        return q_prev * np.ones_like(x)
    q_mm = q_prev * np.ones_like(x)
    q_next = (2 * m + 1) * x * q_mm     # Q[m+1,m]
    if l == m + 1:
        return q_next
    qa, qb = q_mm, q_next
    for ll in range(m + 2, l + 1):
        qc = ((2 * ll - 1) * x * qb - (ll + m - 1) * qa) / (ll - m)
        qa, qb = qb, qc
    return qb


def _poly_cheb_coeffs(l, m_abs, m_is_zero):
    """Chebyshev coefficients (degree<=20) of
    norm_eff * (1-w^2)^floor(m/2) * Q_{l,m}(w)."""
    norm = math.sqrt((2 * l + 1) / (4.0 * math.pi)
                     * math.factorial(l - m_abs) / math.factorial(l + m_abs))
    if not m_is_zero:
        norm *= math.sqrt(2.0)
    t = np.arange(21, dtype=np.float64)
    w = np.cos(np.pi * (t + 0.5) / 21.0)      # Chebyshev nodes
    vals = norm * (1.0 - w * w) ** (m_abs // 2) * _legendre_q(l, m_abs, w)
    return np.polynomial.chebyshev.chebfit(w, vals, MAX_DEGREE)


def _build_rhs(rand_i, rand_j):
    """Two [MM_K, 512] f32 matrices.

    rhsT: cols 0:256 one-hot trig gather for sphere 1 (rows 2*f), cols
    256:512 for sphere 2 (rows 2*f+1).  rhsP: same column split with the
    poly Chebyshev coefficients.  So MM outputs are [TRIG1|TRIG2] and
    [POLY1|POLY2]."""
    rhsT = np.zeros((MM_K, 2 * RANK), dtype=np.float64)
    rhsP = np.zeros((MM_K, 2 * RANK), dtype=np.float64)
    for sphere, rand_idx in ((0, rand_i), (1, rand_j)):
        for r in range(RANK):
            idx = int(rand_idx[r])
            l = int(math.isqrt(idx))
            m = idx - l * l - l
            m_abs = abs(m)
            if m == 0:
                trig_row = 40                 # ones feature
            elif m > 0:
                trig_row = 2 * m - 2          # cos feature
            else:
                trig_row = 2 * m_abs - 1      # sin feature
            rhsT[2 * trig_row + sphere, sphere * RANK + r] = 1.0
            coeffs = _poly_cheb_coeffs(l, m_abs, m == 0)
            for k in range(MAX_DEGREE + 1):
                rhsP[2 * (NTRIG + k) + sphere, sphere * RANK + r] = coeffs[k]
    return rhsT.astype(np.float32), rhsP.astype(np.float32)


# ---------------------------------------------------------------------------
# Device kernel
# ---------------------------------------------------------------------------

def _build_nc():
    nc = bass.Bass("TRN2", target_bir_lowering=False, debug=False)
    coords = nc.declare_dram_parameter("coords", [NPC, 6], F32, isOutput=False)
    rhsA = nc.declare_dram_parameter("rhsA", [MM_K, 2 * RANK], MM_DTYPE,
                                     isOutput=False)
    rhsB = nc.declare_dram_parameter("rhsB", [MM_K, 2 * RANK], MM_DTYPE,
                                     isOutput=False)
    ident = nc.declare_dram_parameter("ident", [P, P], F32, isOutput=False)
    out = nc.declare_dram_parameter("out", [NPC, RANK], F32, isOutput=True)

    # DRAM views
    coords_v = coords.rearrange("(p c) d -> p (c d)", p=P)       # [128, 1536]
    out_v = out.rearrange("(p c) r -> p c r", p=P)               # [128, 256, 256]

    with tile.TileContext(nc) as tc:
        with (
            tc.tile_pool(name="const", bufs=1) as constp,
            tc.tile_pool(name="feat", bufs=1) as featp,
            tc.tile_pool(name="setup", bufs=1) as setupp,
        ):
            # ---- constants -------------------------------------------------
            rA = constp.tile([MM_K, 2 * RANK], MM_DTYPE)
            nc.sync.dma_start(rA[:], rhsA[:])
            rB = constp.tile([MM_K, 2 * RANK], MM_DTYPE)
            nc.sync.dma_start(rB[:], rhsB[:])
            idt = constp.tile([P, P], F32)
            nc.sync.dma_start(idt[:], ident[:])

            # ---- coordinates ----------------------------------------------
            raw = setupp.tile([P, C * 6], F32)
            nc.sync.dma_start(raw[:], coords_v[:])
            raw3 = raw[:].rearrange("p (c d) -> p c d", d=6)

            # Feature regions, one per half of the chunks. Region rows:
            #   0..3   : V_0 chain scratch (o_-1, q_-1, e_0=1, f_0=0)
            #   4t..4t+3 (t=1..10): V_t = (s cos((2t-1)phi), s sin((2t-1)phi),
            #                              cos(2t phi), sin(2t phi))
            #   44     : ones (trig m=0)
            #   45+k   : T_k(w), k=0..21 (row 66 = T_21 spill)
            # Feature j (0..61) lives at region row 4+j; matmul contract row
            # order is (j, sphere).
            NH = 2
            CH = C // NH
            NROWS = 67
            feats = []
            for h in range(NH):
                fh = featp.tile([P, NROWS, 2 * CH], F32, tag=f"feat{h}",
                                name=f"feat{h}")
                feats.append(fh)

            def build_features(h):
                lo = h * CH
                feat = feats[h]
                hFD = 2 * CH

                def row(j):
                    return feat[:, j, :]

                def rows(a, b):
                    return feat[:, a:b, :]

                xb = setupp.tile([P, hFD], F32, tag="xb")
                yb = setupp.tile([P, hFD], F32, tag="yb")
                zb = setupp.tile([P, hFD], F32, tag="zb")
                for sp in range(2):
                    nc.scalar.copy(xb[:, sp * CH:(sp + 1) * CH],
                                   raw3[:, lo:lo + CH, 3 * sp])
                    nc.scalar.copy(yb[:, sp * CH:(sp + 1) * CH],
                                   raw3[:, lo:lo + CH, 3 * sp + 1])
                    nc.scalar.copy(zb[:, sp * CH:(sp + 1) * CH],
                                   raw3[:, lo:lo + CH, 3 * sp + 2])
                x2 = setupp.tile([P, hFD], F32, tag="x2")
                nc.vector.tensor_mul(x2[:], xb[:], xb[:])
                y2 = setupp.tile([P, hFD], F32, tag="y2")
                nc.vector.tensor_mul(y2[:], yb[:], yb[:])
                rho2 = setupp.tile([P, hFD], F32, tag="rho2")
                nc.vector.tensor_add(rho2[:], x2[:], y2[:])
                z2 = setupp.tile([P, hFD], F32, tag="z2")
                nc.vector.tensor_mul(z2[:], zb[:], zb[:])
                r2 = setupp.tile([P, hFD], F32, tag="r2")
                nc.vector.tensor_add(r2[:], rho2[:], z2[:])
                rho2g = setupp.tile([P, hFD], F32, tag="rho2g")
                nc.vector.tensor_scalar_max(rho2g[:], rho2[:], 1e-30)
                rr = setupp.tile([P, hFD], F32, tag="rr")
                nc.scalar.activation(rr[:], r2[:], AF.Sqrt)
                rho = setupp.tile([P, hFD], F32, tag="rho")
                nc.scalar.activation(rho[:], rho2g[:], AF.Sqrt)
                rinv = setupp.tile([P, hFD], F32, tag="rinv")
                nc.vector.reciprocal(rinv[:], rr[:])
                rhoinv = setupp.tile([P, hFD], F32, tag="rhoinv")
                nc.vector.reciprocal(rhoinv[:], rho[:])
                w = setupp.tile([P, hFD], F32, tag="w")
                nc.vector.tensor_mul(w[:], zb[:], rinv[:])
                cphi = setupp.tile([P, hFD], F32, tag="cphi")
                nc.vector.tensor_mul(cphi[:], xb[:], rhoinv[:])
                sphi = setupp.tile([P, hFD], F32, tag="sphi")
                nc.vector.tensor_mul(sphi[:], yb[:], rhoinv[:])
                ssin = setupp.tile([P, hFD], F32, tag="ssin")
                nc.vector.tensor_mul(ssin[:], rho[:], rinv[:])

                # ---- trig chains: V_t = 2cos2phi * V_{t-1} - V_{t-2} ------
                cp2 = setupp.tile([P, hFD], F32, tag="x2")
                nc.vector.tensor_mul(cp2[:], cphi[:], cphi[:])
                c2d = setupp.tile([P, 1, hFD], F32, tag="c2d")
                nc.vector.tensor_scalar(c2d[:, 0, :], cp2[:], 4.0, 2.0,
                                        OP.mult, OP.subtract)
                nc.vector.tensor_scalar(row(6), cp2[:], 2.0, 1.0,
                                        OP.mult, OP.subtract)     # e_1
                tq = setupp.tile([P, hFD], F32, tag="y2")
                nc.vector.tensor_mul(tq[:], cphi[:], sphi[:])
                nc.vector.tensor_add(row(7), tq[:], tq[:])        # f_1
                nc.vector.tensor_mul(row(4), ssin[:], cphi[:])    # o_0
                nc.vector.tensor_mul(row(5), ssin[:], sphi[:])    # q_0
                nc.scalar.copy(row(0), row(4))                    # o_-1
                nc.scalar.mul(row(1), row(5), -1.0)               # q_-1
                nc.gpsimd.memset(row(2), 1.0)                     # e_0
                nc.gpsimd.memset(row(3), 0.0)                     # f_0
                nc.gpsimd.memset(row(44), 1.0)                    # m=0 feature
                c2dB = c2d[:].broadcast_to([P, 4, hFD])
                for t in range(2, 11):
                    tt4 = setupp.tile([P, 4, hFD], F32, tag="tt")
                    nc.vector.tensor_mul(tt4[:], c2dB, rows(4 * t - 4, 4 * t))
                    nc.vector.tensor_sub(rows(4 * t, 4 * t + 4), tt4[:],
                                         rows(4 * t - 8, 4 * t - 4))

                # ---- Chebyshev T_k(w) chains (GPSIMD), double-step --------
                w2t = setupp.tile([P, hFD], F32, tag="z2")
                nc.gpsimd.tensor_mul(w2t[:], w[:], w[:])
                nc.gpsimd.memset(row(45), 1.0)                    # T_0
                nc.gpsimd.tensor_copy(row(46), w[:])              # T_1
                nc.gpsimd.tensor_scalar(row(47), w2t[:], 2.0, 1.0,
                                        OP.mult, OP.subtract)     # T_2
                c2w = setupp.tile([P, 1, hFD], F32, tag="c2w")
                nc.gpsimd.tensor_scalar(c2w[:, 0, :], w2t[:], 4.0, 2.0,
                                        OP.mult, OP.subtract)     # 2 T_2
                ut = setupp.tile([P, hFD], F32, tag="rho2")
                nc.gpsimd.tensor_mul(ut[:], row(47), w[:])
                vt = setupp.tile([P, hFD], F32, tag="r2")
                nc.gpsimd.tensor_add(vt[:], ut[:], ut[:])
                nc.gpsimd.tensor_sub(row(48), vt[:], w[:])        # T_3
                c2wB = c2w[:].broadcast_to([P, 2, hFD])
                for t in range(2, 11):
                    uw = setupp.tile([P, 2, hFD], F32, tag="uw")
                    nc.gpsimd.tensor_mul(uw[:], c2wB,
                                         rows(43 + 2 * t, 45 + 2 * t))
                    nc.gpsimd.tensor_sub(rows(45 + 2 * t, 47 + 2 * t), uw[:],
                                         rows(41 + 2 * t, 43 + 2 * t))

            # ---- per-chunk pipeline ---------------------------------------
            with (
                tc.tile_pool(name="pst", bufs=2, space="PSUM") as pstp,
                tc.tile_pool(name="psT", bufs=2, space="PSUM") as psTp,
                tc.tile_pool(name="psP", bufs=2, space="PSUM") as psPp,
                tc.tile_pool(name="phi", bufs=2) as phip,
                tc.tile_pool(name="trigc", bufs=2) as trigcp,
                tc.tile_pool(name="gfac", bufs=2) as gfacp,
                tc.tile_pool(name="outp", bufs=2) as outp,
            ):
                assert GROUP == 4
                for h in range(NH):
                    build_features(h)
                    for g in range(h * CH // GROUP, (h + 1) * CH // GROUP):
                        gp = gfacp.tile([P, GROUP, 2, RANK], F32)
                        # 4 transposes -> one PSUM bank -> one f32r cast copy
                        pht = pstp.tile([MM_K, GROUP, P], F32)
                        phs = phip.tile([MM_K, GROUP, P], MM_DTYPE)
                        for e in range(GROUP):
                            c = g * GROUP + e
                            cloc = c % CH
                            fin = feats[c // CH][:, 4:66, cloc::CH]  # [128,62,2]
                            nc.tensor.transpose(pht[:, e, :], fin, idt[:])
                        nc.scalar.copy(phs[:], pht[:])
                        for cc2 in range(GROUP // 2):
                            pP2 = psPp.tile([P, 2, 2 * RANK], F32)  # 2 banks
                            tc2 = trigcp.tile([P, 2, 2 * RANK], F32)
                            for e in range(2):
                                ce = cc2 * 2 + e
                                pT = psTp.tile([P, 2 * RANK], F32)
                                nc.tensor.matmul(pT[:], phs[:, ce, :], rA[:],
                                                 start=True, stop=True)
                                nc.tensor.matmul(pP2[:, e, :], phs[:, ce, :],
                                                 rB[:], start=True, stop=True)
                                nc.scalar.copy(tc2[:, e, :], pT[:])
                            nc.vector.tensor_mul(
                                gp[:, cc2 * 2:cc2 * 2 + 2, :, :], tc2[:],
                                pP2[:])
                        ot = outp.tile([P, GROUP * RANK], F32)
                        otv = ot[:].rearrange("p (c r) -> p c r", r=RANK)
                        nc.gpsimd.tensor_mul(otv, gp[:, :, 0, :],
                                             gp[:, :, 1, :])
                        nc.sync.dma_start(
                            out_v[:, g * GROUP:(g + 1) * GROUP, :], ot[:])

    _split_sync_waits(nc)
    return nc


_CACHE = {}


def _install_ntff_shim():
    """Provide antenv.axon_hooks (absent in this image) so that
    run_bass_kernel_spmd(trace=True) can NTFF-profile via the axon .so."""
    import contextlib
    import ctypes
    import sys
    import types

    if "antenv.axon_hooks" in sys.modules:
        return
    so_path = "/opt/axon/libaxon_pjrt.so"
    lib = ctypes.CDLL(so_path)
    lib.axon_start_nrt_profile.argtypes = [
        ctypes.POINTER(ctypes.c_int64), ctypes.c_size_t]
    lib.axon_start_nrt_profile.restype = ctypes.c_int64
    lib.axon_stop_nrt_profile.argtypes = [ctypes.c_char_p]
    lib.axon_stop_nrt_profile.restype = ctypes.c_int64

    @contextlib.contextmanager
    def _hook(output_dir, device_ids):
        import jax
        jax.devices()
        if device_ids:
            ids = (ctypes.c_int64 * len(device_ids))(*device_ids)
            rc = lib.axon_start_nrt_profile(ids, len(device_ids))
        else:
            rc = lib.axon_start_nrt_profile(None, 0)
        if rc != 0:
            raise RuntimeError(f"axon_start_nrt_profile rc={rc}")
        try:
            yield
        finally:
            n = lib.axon_stop_nrt_profile(str(output_dir).encode())
            print(f"ntff profile: {n} file(s) written to {output_dir}")

    mod = types.ModuleType("antenv.axon_hooks")
    mod.get_axon_ntff_profile_hook = lambda: _hook
    mod.set_axon_ntff_profile_hook = lambda h: None
    sys.modules["antenv.axon_hooks"] = mod


def kernel(coordinates, rand_i, rand_j):
    assert coordinates.shape == (N, 6)
    rhsA, rhsB = _build_rhs(np.asarray(rand_i), np.asarray(rand_j))
    identity = np.eye(P, dtype=np.float32)

    if "nc" not in _CACHE:
        _CACHE["nc"] = _build_nc()
    nc = _CACHE["nc"]

    coords = np.ascontiguousarray(coordinates, dtype=np.float32)
    in_maps = []
    for i in range(N_CORES):
        in_maps.append({
            "coords": coords[i * NPC:(i + 1) * NPC],
            "rhsA": rhsA,
            "rhsB": rhsB,
            "ident": identity,
        })
    import os
    trace = bool(os.environ.get("KERNEL_TRACE"))
    if trace:
        _install_ntff_shim()
    res = run_bass_kernel_spmd(nc, in_maps, core_ids=list(range(N_CORES)),
                               trace=trace)
    if trace:
        _CACHE["last_exec_time_ns"] = res.exec_time_ns
        _CACHE["last_profile"] = res
    out = np.concatenate([res.results[i]["out"] for i in range(N_CORES)], axis=0)
    return out.astype(np.float32)
